# revision 1
# baseline (speedup 1.0000x reference)
"""Multi-head attention (B=4, S=2048, d_model=1024, H=16) on 8 TRN2 NeuronCores.

Sharding: core c handles batch c//2 and query rows [1024*(c%2), 1024*(c%2)+1024).
Each core redundantly projects K/V for its batch (no collectives needed) and
produces a disjoint [1024, 1024] slice of the output.

Per-core pipeline:
  phase V: V = v @ w_v + b_v in row layout [t, 16*65] (col 65h+64 := 1.0 so the
           attnV matmul's 65th output row accumulates sum(exp(scores)) for free)
  phase KQ: KT/QT in channel-major layout per head pair (fp32r matmuls)
  attention (per pair, per 512-query block): scoresT = K_h @ Q_h^T with the two
           heads of a pair run concurrently on disjoint PE row groups (K=64);
           exp on ScalarE (1/sqrt(d_k) folded into the activation scale);
           attnV with M=65 accumulating over 16 key chunks; unnormalized
           outputs + softmax denominators spill to DRAM
  epilogue: one dense 128-lane reciprocal of all 16K denominators (via DRAM
           reshape bounce); K=2 ones-matmul broadcasts recips across
           partitions; normalize; out-projection (fp32r) + bias; DMA out.
"""

import numpy as np

import bass_rust
import concourse.bass as bass
import concourse.mybir as mybir
import concourse.tile as tile
from concourse.bass_utils import run_bass_kernel_spmd
from concourse.vector_clock import ScopedClock

F32 = mybir.dt.float32
F32R = mybir.dt.float32r
BF16 = mybir.dt.bfloat16
AF = mybir.ActivationFunctionType
ADD = mybir.AluOpType.add
MULT = mybir.AluOpType.mult

D_MODEL = 1024
B = 4
S = 2048
N_CORES = 8
QL = 1024  # query rows per core
NPAIR = 8  # head pairs
NK = D_MODEL // 128  # contraction chunks
NT = S // 128  # key chunks
VPW = 65 * 16  # padded V width

# ---------------------------------------------------------------------------
# Workaround for this container's walrus build: each instruction may carry at
# most ONE embedded sync-wait ("Too many sync wait commands" otherwise). Tile
# attaches several; split the extras onto same-engine NOPs placed immediately
# before the instruction (engine queues are in-order => identical semantics).
_MAX_WAITS = 1


def _patched_lower(self, ordered):
    nc = self.nc
    for bb_name, insts in ordered.items():
        new_list = []
        for inst in insts:
            si = inst.sync_info
            waits = list(si.on_wait) if si is not None and si.on_wait else []
            if len(waits) > _MAX_WAITS:
                updates = list(si.on_update) if si.on_update else []
                for w in waits[:-_MAX_WAITS]:
                    nop = bass_rust.InstNoOp(
                        name=nc.get_next_instruction_name(),
                        engine=inst.engine,
                        debug=inst.debug,
                        sync_info=bass_rust.SyncInfo(on_wait=[w], on_update=[]),
                    )
                    new_list.append(nop)
                inst.sync_info = bass_rust.SyncInfo(
                    on_wait=waits[-_MAX_WAITS:], on_update=updates
                )
            new_list.append(inst)
        insts[:] = new_list
    return tile.TileContext._orig_lower_ordered_insts(self, ordered)


def _patched_drain(self, tick_clock, wait_clock):
    probe = self.nc.sync.nop(nofuse=True)
    wait_clock.add_sem_waits(probe.ins, ScopedClock({None: tick_clock.global_clock}))
    si = probe.ins.sync_info
    waits = list(si.on_wait) if si is not None and si.on_wait else []
    if len(waits) > _MAX_WAITS:
        probe.ins.sync_info = bass_rust.SyncInfo(
            on_wait=waits[:_MAX_WAITS], on_update=[]
        )
        for w in waits[_MAX_WAITS:]:
            n = self.nc.sync.nop(nofuse=True)
            n.ins.sync_info = bass_rust.SyncInfo(on_wait=[w], on_update=[])
    self.nc.sync.drain()
    self.nc.all_engine_barrier()
    assert self.sems is not None
    popped = self.nc._tile_sem_poison_stack.pop()
    assert popped is self._sem_poison
    self.nc.clear_and_free_semaphores(list(self.sems.allocated().values()))
    self.nc.all_engine_barrier()




import concourse.bass_utils as _bu


def _bvo_ldwopt(
    tmpdir, inp="bir.json", outp="file.neff", arch=None, *, dve_root=None
):
    cmd = [
        _bu.get_walrus_driver(),
        "--pass",
        ",".join(
            [
                "birverifier",
                "runtime_memory_reservation",
                "lower_act",
                "lower_dve",
                "lower_ap_offset",
                "codegen",
                "neff_packager",
            ]
        ),
        "-i",
        inp,
        "--neff-output-filename",
        outp,
        "--enable-birsim=true",
        "--mem-mode=physical",
        "--policy=0",
        "--enable-ldw-opt=true",
        "--assign-static-dmas-to-sp=false",
        f"--dram-page-size={_bu.aot_getenv('NEURON_SCRATCHPAD_PAGE_SIZE', '256')}",
        "--enable-neff-debug-info=true",
        "--jobs",
        "8",
        *_bu.get_walrus_args(
            _bu.get_bir_arch(tmpdir, inp) if arch is None else arch,
            tmpdir,
            dve_root=dve_root,
        ),
    ]
    result = _bu.run_command(cmd, cwd=tmpdir)
    if result is not None:
        (_bu.Path(tmpdir) / "log.txt").write_text(result.stdout)
    return f"{tmpdir}/{outp}"


def _install_ldwopt():
    import os

    # ldw-opt splits fp32r matmuls into standalone InstLdweights, which this
    # walrus cannot codegen (the reason the flag ships disabled). Opt-in only.
    if os.environ.get("USE_LDW_OPT") == "1":
        _bu.bir_verify_and_optimise = _bvo_ldwopt


def _install_patch():
    _install_ldwopt()
    if not hasattr(tile.TileContext, "_orig_lower_ordered_insts"):
        tile.TileContext._orig_lower_ordered_insts = (
            tile.TileContext._lower_ordered_insts
        )
        tile.TileContext._lower_ordered_insts = _patched_lower
        tile.TileContext._drain_and_barrier = _patched_drain


# ---------------------------------------------------------------------------


def _build_bass():
    nc = bass.Bass()
    qt = nc.dram_tensor("qt", [D_MODEL, QL], BF16, kind="ExternalInput")
    kt = nc.dram_tensor("kt", [D_MODEL, S], BF16, kind="ExternalInput")
    vt = nc.dram_tensor("vt", [NT, 128, 1024], BF16, kind="ExternalInput")
    wq = nc.dram_tensor("wq", [NPAIR, NK, 128, 128], BF16, kind="ExternalInput")
    wk = nc.dram_tensor("wk", [NPAIR, NK, 128, 128], BF16, kind="ExternalInput")
    wv = nc.dram_tensor("wv", [D_MODEL, D_MODEL], BF16, kind="ExternalInput")
    wo = nc.dram_tensor("wo", [D_MODEL, D_MODEL], BF16, kind="ExternalInput")
    bqt = nc.dram_tensor("bqt", [128, NK], F32, kind="ExternalInput")
    bkt = nc.dram_tensor("bkt", [128, NK], F32, kind="ExternalInput")
    bvr = nc.dram_tensor("bvr", [128, D_MODEL], F32, kind="ExternalInput")
    bor = nc.dram_tensor("bor", [128, D_MODEL], F32, kind="ExternalInput")
    ones2 = nc.dram_tensor("ones2", [128, 128], F32R, kind="ExternalInput")
    vones = nc.dram_tensor("vones", [128, 16], BF16, kind="ExternalInput")
    out = nc.dram_tensor("out", [QL, D_MODEL], F32, kind="ExternalOutput")
    xau = nc.dram_tensor("xau", [D_MODEL, QL], F32)  # unnormalized X_attn^T
    sums_d = nc.dram_tensor("sums_d", [128, 128], F32)
    sums_r = nc.dram_tensor("sums_r", [128, 128], F32)

    with tile.TileContext(nc) as tc:
        _emit(nc, tc, locals())
    return nc


def _emit(nc, tc, t):
    qt, kt, vt = t["qt"], t["kt"], t["vt"]
    wq, wk, wv, wo = t["wq"], t["wk"], t["wv"], t["wo"]
    bqt, bkt, bvr, bor = t["bqt"], t["bkt"], t["bvr"], t["bor"]
    ones2, out, vones = t["ones2"], t["out"], t["vones"]
    xau, sums_d, sums_r = t["xau"], t["sums_d"], t["sums_r"]

    P = tc.tile_pool

    with (
        P(name="consts", bufs=1) as consts,
        P(name="stg", bufs=2) as stg,
    ):
        ones_t = consts.tile([128, 128], F32R, tag="ones2")
        nc.sync.dma_start(ones_t[:], ones2[:])
        bqt_t = consts.tile([128, NK], F32, tag="bqt")
        nc.sync.dma_start(bqt_t[:], bqt[:])
        bkt_t = consts.tile([128, NK], F32, tag="bkt")
        nc.sync.dma_start(bkt_t[:], bkt[:])
        bvr_t = consts.tile([128, D_MODEL], F32, tag="bvr")
        nc.sync.dma_start(bvr_t[:], bvr[:])
        bor_t = consts.tile([128, D_MODEL], F32, tag="bor")
        nc.sync.dma_start(bor_t[:], bor[:])

        with P(name="pv", bufs=1) as pv, P(name="pkq", bufs=1) as pkq:
            # ---- V projection (bf16): V_pad [t, 16*65] row-major ---------
            v_tiles = []
            for c in range(NT):
                v = pv.tile([128, VPW], BF16, name=f"v{c}", tag=f"v{c}")
                v_tiles.append(v)

            with (
                P(name="wvp", bufs=1) as wvp,
                P(name="vstr", bufs=5) as vstr,
                P(name="psV", bufs=3, space="PSUM") as psV,
            ):
                wv_tiles = []
                for k in range(NK):
                    wvt = wvp.tile([128, D_MODEL], BF16, name=f"wv{k}", tag=f"wv{k}")
                    nc.sync.dma_start(wvt[:], wv[128 * k : 128 * k + 128, :])
                    wv_tiles.append(wvt)
                for c in range(NT):
                    vts = vstr.tile([128, 1024], BF16, tag="vts")
                    nc.sync.dma_start(vts[:], vt[c])
                    ps = psV.tile([128, 1024], F32, tag="vproj")
                    for k in range(NK):
                        for j in range(2):
                            nc.tensor.matmul(
                                ps[:, 512 * j : 512 * j + 512],
                                vts[:, 128 * k : 128 * k + 128],
                                wv_tiles[k][:, 512 * j : 512 * j + 512],
                                start=(k == 0),
                                stop=(k == NK - 1),
                                skip_group_check=True,
                            )
                    dst = v_tiles[c][:, :].rearrange("p (h w) -> p h w", w=65)[
                        :, :, 0:64
                    ]
                    nc.vector.tensor_tensor(
                        dst,
                        ps[:, :].rearrange("p (h w) -> p h w", w=64),
                        bvr_t[:, :].rearrange("p (h w) -> p h w", w=64),
                        ADD,
                    )

            for c in range(NT):
                nc.gpsimd.dma_start(
                    v_tiles[c][:, :].rearrange("p (h w) -> p h w", w=65)[:, :, 64:65],
                    vones[:, :, None],
                )

            # ---- K/Q projections (bf16) interleaved into attention -------
            sums_flat = sums_d[:, :].rearrange("p f -> (p f)")
            sums_r_flat = sums_r[:, :].rearrange("p f -> (p f)")
            xn_tiles = [None] * NPAIR
            KT = [
                pkq.tile([128, S], BF16, name=f"ktg{g}", tag=f"ktg{g}")
                for g in range(NPAIR)
            ]
            QT = [
                pkq.tile([128, QL], BF16, name=f"qtg{g}", tag=f"qtg{g}")
                for g in range(NPAIR)
            ]
            pxn = None  # set below; must outlive into the out-projection

            with (
                P(name="kstr", bufs=1) as kstr,
                P(name="qstr", bufs=1) as qstr,
                P(name="wks", bufs=2) as wks,
                P(name="wqs", bufs=2) as wqs,
                P(name="expp", bufs=6) as expp,
                P(name="psS", bufs=2, space="PSUM") as psS,
                P(name="psacc", bufs=1, space="PSUM") as psacc,
                P(name="psP", bufs=1, space="PSUM") as psP,
                P(name="ph3s", bufs=2) as ph3s,
            ):
                pxn = pkq  # xn tiles live in the long-lived pkq pool
                kfull = []
                for k in range(NK):
                    ktile = kstr.tile([128, S], BF16, name=f"ktf{k}", tag=f"ktf{k}")
                    nc.sync.dma_start(ktile[:], kt[128 * k : 128 * k + 128, :])
                    kfull.append(ktile)
                qfull = []
                for k in range(NK):
                    qtile = qstr.tile([128, QL], BF16, name=f"qtf{k}", tag=f"qtf{k}")
                    nc.sync.dma_start(qtile[:], qt[128 * k : 128 * k + 128, :])
                    qfull.append(qtile)

                def emit_kproj(half, g):
                    wkg = []
                    for k in range(NK):
                        wkt = wks.tile([128, 128], BF16, tag=f"wks{k}")
                        nc.sync.dma_start(wkt[:], wk[g, k])
                        wkg.append(wkt)
                    ps = psP.tile([128, 1024], F32, tag="kproj")
                    for k in range(NK):
                        for j in range(2):
                            nc.tensor.matmul(
                                ps[:, 512 * j : 512 * j + 512],
                                wkg[k][:],
                                kfull[k][
                                    :,
                                    1024 * half + 512 * j : 1024 * half + 512 * j + 512,
                                ],
                                start=(k == 0),
                                stop=(k == NK - 1),
                                skip_group_check=True,
                            )
                    nc.vector.tensor_scalar_add(
                        KT[g][:, 1024 * half : 1024 * half + 1024],
                        ps[:],
                        bkt_t[:, g : g + 1],
                    )

                def emit_qproj(g):
                    wqg = []
                    for k in range(NK):
                        wqt = wqs.tile([128, 128], BF16, tag=f"wqs{k}")
                        nc.sync.dma_start(wqt[:], wq[g, k])
                        wqg.append(wqt)
                    ps = psP.tile([128, 1024], F32, tag="kproj")
                    for k in range(NK):
                        for j in range(2):
                            nc.tensor.matmul(
                                ps[:, 512 * j : 512 * j + 512],
                                wqg[k][:],
                                qfull[k][:, 512 * j : 512 * j + 512],
                                start=(k == 0),
                                stop=(k == NK - 1),
                                skip_group_check=True,
                            )
                    nc.vector.tensor_scalar_add(QT[g][:], ps[:], bqt_t[:, g : g + 1])

                for g in range(3):
                    emit_kproj(0, g)
                    emit_kproj(1, g)
                    emit_qproj(g)

                proj_steps = []
                for g in range(3, NPAIR):
                    proj_steps.append((emit_kproj, (0, g)))
                    proj_steps.append((emit_kproj, (1, g)))
                    proj_steps.append((emit_qproj, (g,)))
                proj_iter = iter(proj_steps)

                def emit_scores(g, qb, cg):
                    ktg, qtg = KT[g], QT[g]
                    q0 = 512 * qb
                    tiles = []
                    for h in range(2):
                        p0 = 64 * h
                        sc = psS.tile([128, QL], F32, tag="scores")
                        for ci in range(2):
                            c = 2 * cg + ci
                            nc.tensor.matmul(
                                sc[:, 512 * ci : 512 * ci + 512],
                                ktg[p0 : p0 + 64, 128 * c : 128 * c + 128],
                                qtg[p0 : p0 + 64, q0 : q0 + 512],
                                start=True,
                                stop=True,
                                skip_group_check=True,
                            )
                        tiles.append(sc)
                    return tiles

                def emit_spill(g, qb, acc):
                    q0 = 512 * qb
                    for h in range(2):
                        sg = stg.tile([65, 512], F32, tag="spill")
                        nc.vector.tensor_copy(sg[:], acc[h][0:65, :])
                        nc.sync.dma_start(
                            xau[
                                128 * g + 64 * h : 128 * g + 64 * h + 64,
                                q0 : q0 + 512,
                            ],
                            sg[0:64, :],
                        )
                        base = g * 2048 + h * 1024 + 512 * qb
                        nc.sync.dma_start(
                            sums_flat[base : base + 512][None, :], sg[64:65, :]
                        )

                def emit_recip_batch(glo, ghi):
                    r0, r1 = 16 * glo, 16 * ghi
                    den = stg.tile([16 * (ghi - glo), 128], F32, tag="dense")
                    nc.sync.dma_start(den[:], sums_d[r0:r1, :])
                    denr = stg.tile([16 * (ghi - glo), 128], F32, tag="denser")
                    nc.vector.reciprocal(denr[:], den[:])
                    nc.sync.dma_start(sums_r[r0:r1, :], denr[:])

                def emit_chain(g):
                    srr = ph3s.tile([128, QL], F32R, tag="srr")
                    for h in range(2):
                        base = g * 2048 + h * 1024
                        nc.gpsimd.dma_start(
                            srr[64 + h : 65 + h, :],
                            sums_r_flat[base : base + QL][None, :],
                        )
                    xr = ph3s.tile([128, QL], F32, tag="xr")
                    nc.sync.dma_start(xr[:], xau[128 * g : 128 * g + 128, :])
                    rep = psP.tile([128, QL], F32, tag="kproj")
                    for j in range(2):
                        nc.tensor.matmul(
                            rep[:, 512 * j : 512 * j + 512],
                            ones_t[64:66, :],
                            srr[64:66, 512 * j : 512 * j + 512],
                            start=True,
                            stop=True,
                            skip_group_check=True,
                        )
                    xn = pxn.tile([128, QL], BF16, name=f"xn{g}", tag=f"xn{g}")
                    nc.vector.tensor_tensor(xn[:], xr[:], rep[:], MULT)
                    xn_tiles[g] = xn

                chain_iter = iter(range(6))
                pending_spill = None
                slot = 0
                for g in range(NPAIR):
                    for qb in range(2):
                        acc = [
                            psacc.tile([65, 512], F32, name="acca", tag="acca"),
                            psacc.tile([65, 512], F32, name="accb", tag="accb"),
                        ]
                        sc_cur = emit_scores(g, qb, 0)
                        if pending_spill is not None:
                            emit_spill(*pending_spill)
                            pending_spill = None
                            if g == 6 and qb == 0:
                                emit_recip_batch(0, 6)
                        for cg in range(NT // 2):
                            sc_next = (
                                emit_scores(g, qb, cg + 1)
                                if cg + 1 < NT // 2
                                else None
                            )
                            for h in range(2):
                                hh = 2 * g + h
                                ex = expp.tile([128, QL], BF16, tag="exp")
                                nc.scalar.activation(
                                    ex[:], sc_cur[h][:], AF.Exp, scale=0.125
                                )
                                for ci in range(2):
                                    c = 2 * cg + ci
                                    nc.tensor.matmul(
                                        acc[h][:],
                                        v_tiles[c][:, 65 * hh : 65 * hh + 65],
                                        ex[:, 512 * ci : 512 * ci + 512],
                                        start=(c == 0),
                                        stop=(c == NT - 1),
                                        skip_group_check=True,
                                    )
                            if g >= 2 and slot % 3 == 0:
                                step = next(proj_iter, None)
                                if step is not None:
                                    step[0](*step[1])
                            if 2 * g + qb >= 13 and slot % 4 == 0:
                                cidx = next(chain_iter, None)
                                if cidx is not None:
                                    emit_chain(cidx)
                            if g >= 2:
                                slot += 1
                            sc_cur = sc_next
                        pending_spill = (g, qb, acc)
                emit_spill(*pending_spill)
                for step in proj_iter:
                    step[0](*step[1])
                emit_recip_batch(6, NPAIR)
                for cidx in chain_iter:
                    emit_chain(cidx)
                emit_chain(6)
                emit_chain(7)

        # ---- output projection (bf16) ------------------------------------
        with (
            P(name="pwo", bufs=1) as pwo,
            P(name="ps3o", bufs=2, space="PSUM") as ps3o,
        ):
            wo_tiles = []
            for k in range(NK):
                wot = pwo.tile([128, D_MODEL], BF16, name=f"wo{k}", tag=f"wo{k}")
                nc.sync.dma_start(wot[:], wo[128 * k : 128 * k + 128, :])
                wo_tiles.append(wot)

            for m in range(QL // 128):
                ps = ps3o.tile([128, D_MODEL], F32, tag="oproj")
                for g in range(NPAIR):
                    for j in range(2):
                        nc.tensor.matmul(
                            ps[:, 512 * j : 512 * j + 512],
                            xn_tiles[g][:, 128 * m : 128 * m + 128],
                            wo_tiles[g][:, 512 * j : 512 * j + 512],
                            start=(g == 0),
                            stop=(g == NPAIR - 1),
                            skip_group_check=True,
                        )
                ot = stg.tile([128, D_MODEL], F32, tag="outs")
                nc.vector.tensor_tensor(ot[:], ps[:], bor_t[:], ADD)
                nc.sync.dma_start(out[128 * m : 128 * m + 128, :], ot[:])


_NC_CACHE = None
LAST_RESULT = None


def _get_nc():
    global _NC_CACHE
    if _NC_CACHE is None:
        _install_patch()
        _NC_CACHE = _build_bass()
    return _NC_CACHE


def kernel(q, k, v, w_q, b_q, w_k, b_k, w_v, b_v, w_o, b_o):
    global LAST_RESULT
    import ml_dtypes

    q = np.asarray(q, np.float32)
    k = np.asarray(k, np.float32)
    v = np.asarray(v, np.float32)
    def _tile_w(w):
        # [in, out] -> [g, k, 128, 128] contiguous tiles: w[128k:+128, 128g:+128]
        return np.ascontiguousarray(
            np.asarray(w, np.float32)
            .reshape(NK, 128, NPAIR, 128)
            .transpose(2, 0, 1, 3)
        ).astype(ml_dtypes.bfloat16)

    w_q = _tile_w(w_q)
    w_k = _tile_w(w_k)
    w_v = np.asarray(w_v, np.float32).astype(ml_dtypes.bfloat16)
    w_o = np.asarray(w_o, np.float32).astype(ml_dtypes.bfloat16)
    b_q = np.asarray(b_q, np.float32)
    b_k = np.asarray(b_k, np.float32)
    b_v = np.asarray(b_v, np.float32)
    b_o = np.asarray(b_o, np.float32)

    bqt = np.ascontiguousarray(b_q.reshape(NK, 128).T)
    bkt = np.ascontiguousarray(b_k.reshape(NK, 128).T)
    bvr = np.ascontiguousarray(np.broadcast_to(b_v[None, :], (128, D_MODEL)))
    bor = np.ascontiguousarray(np.broadcast_to(b_o[None, :], (128, D_MODEL)))
    ones2 = np.zeros((128, 128), np.float32)
    ones2[64, 0:64] = 1.0
    ones2[65, 64:128] = 1.0
    vones_np = np.ones((128, 16), ml_dtypes.bfloat16)

    in_maps = []
    for c in range(N_CORES):
        b = c // 2
        r0 = QL * (c % 2)
        in_maps.append(
            {
                "qt": np.ascontiguousarray(q[b, r0 : r0 + QL, :].T).astype(
                    ml_dtypes.bfloat16
                ),
                "kt": np.ascontiguousarray(k[b].T).astype(ml_dtypes.bfloat16),
                "vt": np.ascontiguousarray(
                    v[b]
                    .T.reshape(8, 128, 16, 128)
                    .transpose(2, 1, 0, 3)
                    .reshape(16, 128, 1024)
                ).astype(ml_dtypes.bfloat16),
                "wq": w_q,
                "wk": w_k,
                "wv": w_v,
                "wo": w_o,
                "bqt": bqt,
                "bkt": bkt,
                "bvr": bvr,
                "bor": bor,
                "ones2": ones2,
                "vones": vones_np,
            }
        )

    nc = _get_nc()
    res = run_bass_kernel_spmd(nc, in_maps, list(range(N_CORES)))
    LAST_RESULT = res

    outp = np.empty((B, S, D_MODEL), np.float32)
    for c in range(N_CORES):
        b = c // 2
        r0 = QL * (c % 2)
        outp[b, r0 : r0 + QL, :] = res.results[c]["out"]
    return outp



# revision 28
# speedup vs baseline: 1.2230x; 1.2230x over previous
"""Multi-head attention (B=4, S=2048, d_model=1024, H=16) on 8 TRN2 NeuronCores.

Sharding: core c handles batch c//2 and query rows [1024*(c%2), +1024).
Each core redundantly projects K/V for its batch (no collectives) and
produces a disjoint [1024, 1024] slice of the output.

v2 structure (vs baseline):
  - every K=128 matmul is split into two K=64 row-group matmuls emitted
    alternately, so each LDWEIGHTS targets the row half not currently
    streaming (PE pulls it ahead; halves can also run concurrently).
  - input DMAs use 2KB+ rows and are spread across sync/scalar/vector/
    gpsimd queues; ones-columns of V are memset, not DMAed.
  - unnormalized attention output stays in SBUF (bf16); softmax sums go
    to tiny per-pair den tiles; recip broadcast via fp16 selector matmul.
  - projections for g>=1 interleave into the attention loop from g=0;
    per-pair chains run as soon as a pair's spills land.
"""

import numpy as np

import bass_rust
import concourse.bass as bass
import concourse.mybir as mybir
import concourse.tile as tile
from concourse.bass_utils import run_bass_kernel_spmd
from concourse.vector_clock import ScopedClock

F32 = mybir.dt.float32
F16 = mybir.dt.float16
FP8 = mybir.dt.float8e4
BF16 = mybir.dt.bfloat16
AF = mybir.ActivationFunctionType
ADD = mybir.AluOpType.add
MULT = mybir.AluOpType.mult

D_MODEL = 1024
B = 4
S = 2048
N_CORES = 8
QL = 1024  # query rows per core
NPAIR = 8  # head pairs
NK = D_MODEL // 128  # contraction chunks
NT = S // 128  # key chunks
VPW = 65 * 16  # padded V width
ROW_SPLIT = False  # split K=128 matmuls into two K=64 row-group matmuls
ATTNV_FP8 = False  # attnV via fp8e4 DoubleRow (chunk-pairs, exp shifted by -3)


def _rhs():
    return ((0, 64), (64, 128)) if ROW_SPLIT else ((0, 128),)

# ---------------------------------------------------------------------------
# Workaround for this container's walrus build: each instruction may carry at
# most ONE embedded sync-wait ("Too many sync wait commands" otherwise). Tile
# attaches several; split the extras onto same-engine NOPs placed immediately
# before the instruction (engine queues are in-order => identical semantics).
_MAX_WAITS = 1


def _patched_lower(self, ordered):
    nc = self.nc
    for bb_name, insts in ordered.items():
        new_list = []
        for inst in insts:
            si = inst.sync_info
            waits = list(si.on_wait) if si is not None and si.on_wait else []
            if len(waits) > _MAX_WAITS:
                updates = list(si.on_update) if si.on_update else []
                for w in waits[:-_MAX_WAITS]:
                    nop = bass_rust.InstNoOp(
                        name=nc.get_next_instruction_name(),
                        engine=inst.engine,
                        debug=inst.debug,
                        sync_info=bass_rust.SyncInfo(on_wait=[w], on_update=[]),
                    )
                    new_list.append(nop)
                inst.sync_info = bass_rust.SyncInfo(
                    on_wait=waits[-_MAX_WAITS:], on_update=updates
                )
            new_list.append(inst)
        insts[:] = new_list
    return tile.TileContext._orig_lower_ordered_insts(self, ordered)


def _patched_drain(self, tick_clock, wait_clock):
    probe = self.nc.sync.nop(nofuse=True)
    wait_clock.add_sem_waits(probe.ins, ScopedClock({None: tick_clock.global_clock}))
    si = probe.ins.sync_info
    waits = list(si.on_wait) if si is not None and si.on_wait else []
    if len(waits) > _MAX_WAITS:
        probe.ins.sync_info = bass_rust.SyncInfo(
            on_wait=waits[:_MAX_WAITS], on_update=[]
        )
        for w in waits[_MAX_WAITS:]:
            n = self.nc.sync.nop(nofuse=True)
            n.ins.sync_info = bass_rust.SyncInfo(on_wait=[w], on_update=[])
    self.nc.sync.drain()
    self.nc.all_engine_barrier()
    assert self.sems is not None
    popped = self.nc._tile_sem_poison_stack.pop()
    assert popped is self._sem_poison
    self.nc.clear_and_free_semaphores(list(self.sems.allocated().values()))
    self.nc.all_engine_barrier()


import concourse.bass_utils as _bu


def _install_patch():
    if not hasattr(tile.TileContext, "_orig_lower_ordered_insts"):
        tile.TileContext._orig_lower_ordered_insts = (
            tile.TileContext._lower_ordered_insts
        )
        tile.TileContext._lower_ordered_insts = _patched_lower
        tile.TileContext._drain_and_barrier = _patched_drain


# ---------------------------------------------------------------------------


def _build_bass():
    nc = bass.Bass()
    qt = nc.dram_tensor("qt", [128, NK * QL], BF16, kind="ExternalInput")
    kt = nc.dram_tensor("kt", [128, NK * S], BF16, kind="ExternalInput")
    vt = nc.dram_tensor("vt", [NT, 128, 1024], BF16, kind="ExternalInput")
    wq = nc.dram_tensor("wq", [NPAIR, 128, D_MODEL], BF16, kind="ExternalInput")
    wk = nc.dram_tensor("wk", [NPAIR, 128, D_MODEL], BF16, kind="ExternalInput")
    wv = nc.dram_tensor("wv", [128, NK * D_MODEL], BF16, kind="ExternalInput")
    wo = nc.dram_tensor("wo", [128, NPAIR * D_MODEL], BF16, kind="ExternalInput")
    bqt = nc.dram_tensor("bqt", [128, NK], F32, kind="ExternalInput")
    bkt = nc.dram_tensor("bkt", [128, NK], F32, kind="ExternalInput")
    bvr = nc.dram_tensor("bvr", [128, D_MODEL], BF16, kind="ExternalInput")
    bor = nc.dram_tensor("bor", [128, D_MODEL], BF16, kind="ExternalInput")
    selq = nc.dram_tensor("selq", [2, 4, 128], F16, kind="ExternalInput")
    out = nc.dram_tensor("out", [QL, D_MODEL], BF16, kind="ExternalOutput")

    with tile.TileContext(nc) as tc:
        _emit(nc, tc, locals())
    return nc


def _emit(nc, tc, t):
    qt, kt, vt = t["qt"], t["kt"], t["vt"]
    wq, wk, wv, wo = t["wq"], t["wk"], t["wv"], t["wo"]
    bqt, bkt, bvr, bor = t["bqt"], t["bkt"], t["bvr"], t["bor"]
    selq, out = t["selq"], t["out"]

    P = tc.tile_pool

    with (
        P(name="consts", bufs=1) as consts,
        P(name="stg", bufs=3) as stg,
        P(name="sel", bufs=1) as selp,
        P(name="den", bufs=3) as denp,
    ):
        bqt_t = consts.tile([128, NK], F32, tag="bqt")
        nc.gpsimd.dma_start(bqt_t[:], bqt[:])
        bkt_t = consts.tile([128, NK], F32, tag="bkt")
        nc.gpsimd.dma_start(bkt_t[:], bkt[:])
        bvr_t = consts.tile([128, D_MODEL], BF16, tag="bvr")
        nc.gpsimd.dma_start(bvr_t[:], bvr[:])

        # selector tiles for the recip broadcast: sel[qb][r, p] = 1 iff
        # r == 2*(p//64) + qb  (fp16, host constant)
        sel = []
        for qb in range(2):
            st = selp.tile([4, 128], F16, name=f"sel{qb}", tag=f"sel{qb}")
            nc.gpsimd.dma_start(st[:], selq[qb])
            sel.append(st)
        expb = selp.tile([128, 1], F32, tag="expb")
        nc.gpsimd.memset(expb[:], -3.0)
        # per-pair softmax-denominator tiles [4, 512] (row r = 2*h + qb)
        den = [None] * NPAIR
        denr = [None] * NPAIR

        with P(name="pv", bufs=1) as pv, P(name="pkq", bufs=1) as pkq:
            # ---- V projection: V_pad [t, 16*65] row-major, bf16 ----------
            VDT = FP8 if ATTNV_FP8 else BF16
            if ATTNV_FP8:
                # chunk-PAIR tiles [128, 2*VPW] for DoubleRow ([Ki, Ko=2, M])
                vp_tiles = [
                    pv.tile([128, 2 * VPW], FP8, name=f"v{cp}", tag=f"v{cp}")
                    for cp in range(NT // 2)
                ]
                def vslice(c):
                    return vp_tiles[c // 2][
                        :, VPW * (c % 2) : VPW * (c % 2) + VPW
                    ]
            else:
                v_tiles = [
                    pv.tile([128, VPW], BF16, name=f"v{c}", tag=f"v{c}")
                    for c in range(NT)
                ]
                def vslice(c):
                    return v_tiles[c][:, :]
            # ones columns (col 65h+64) provide softmax sums in attnV
            for c in range(NT):
                nc.gpsimd.memset(
                    vslice(c).rearrange("p (h w) -> p h w", w=65)[:, :, 64:65],
                    1.0,
                )

            with (
                P(name="wvp", bufs=1) as wvp,
                P(name="vstr", bufs=5) as vstr,
                P(name="psV", bufs=3, space="PSUM") as psV,
            ):
                wv_sb = wvp.tile([128, NK * D_MODEL], BF16, tag="wvsb")
                nc.scalar.dma_start(wv_sb[:, 0:5120], wv[:, 0:5120])
                nc.gpsimd.dma_start(wv_sb[:, 5120:8192], wv[:, 5120:8192])
                for c in range(NT):
                    vts = vstr.tile([128, 1024], BF16, tag="vts")
                    nc.sync.dma_start(vts[:], vt[c])
                    ps = psV.tile([128, 1024], F32, tag="vproj")
                    for k in range(NK):
                        for j in range(2):
                            for r0, r1 in _rhs():
                                nc.tensor.matmul(
                                    ps[:, 512 * j : 512 * j + 512],
                                    vts[r0:r1, 128 * k : 128 * k + 128],
                                    wv_sb[
                                        r0:r1,
                                        1024 * k + 512 * j : 1024 * k + 512 * j + 512,
                                    ],
                                    start=(k == 0 and r0 == 0),
                                    stop=(k == NK - 1 and r1 == 128),
                                    skip_group_check=True,
                                )
                    dst = vslice(c).rearrange("p (h w) -> p h w", w=65)[:, :, 0:64]
                    with nc.allow_low_precision(
                        reason="fp8 V for DoubleRow attnV; softmax-averaged"
                    ):
                        nc.vector.tensor_tensor(
                            dst,
                            ps[:, :].rearrange("p (h w) -> p h w", w=64),
                            bvr_t[:, :].rearrange("p (h w) -> p h w", w=64),
                            ADD,
                        )

            # ---- K/Q projections interleaved into attention --------------
            xn_tiles = [None] * NPAIR
            xg_tiles = [None] * NPAIR
            KT = [
                pkq.tile([128, S], BF16, name=f"ktg{g}", tag=f"ktg{g}")
                for g in range(NPAIR)
            ]
            QT = [
                pkq.tile([128, QL], BF16, name=f"qtg{g}", tag=f"qtg{g}")
                for g in range(NPAIR)
            ]

            import contextlib

            def emit_kproj(half, g, wkg):
                # KT[g][:, 1024*half:+1024] = (wk[g].T @ K^T)(half) + bias
                ps = psP.tile([128, 1024], F32, tag="kproj")
                for k in range(NK):
                    for j in range(2):
                        for r0, r1 in _rhs():
                            nc.tensor.matmul(
                                ps[:, 512 * j : 512 * j + 512],
                                wkg[r0:r1, 128 * k : 128 * k + 128],
                                kt_sb[
                                    r0:r1,
                                    2048 * k
                                    + 1024 * half
                                    + 512 * j : 2048 * k
                                    + 1024 * half
                                    + 512 * j
                                    + 512,
                                ],
                                start=(k == 0 and r0 == 0),
                                stop=(k == NK - 1 and r1 == 128),
                                skip_group_check=True,
                            )
                nc.vector.tensor_scalar_add(
                    KT[g][:, 1024 * half : 1024 * half + 1024],
                    ps[:],
                    bkt_t[:, g : g + 1],
                )

            def emit_qproj(g, wqg):
                ps = psP.tile([128, 1024], F32, tag="kproj")
                for k in range(NK):
                    for j in range(2):
                        for r0, r1 in _rhs():
                            nc.tensor.matmul(
                                ps[:, 512 * j : 512 * j + 512],
                                wqg[r0:r1, 128 * k : 128 * k + 128],
                                qt_sb[
                                    r0:r1,
                                    1024 * k + 512 * j : 1024 * k + 512 * j + 512,
                                ],
                                start=(k == 0 and r0 == 0),
                                stop=(k == NK - 1 and r1 == 128),
                                skip_group_check=True,
                            )
                nc.vector.tensor_scalar_add(QT[g][:], ps[:], bqt_t[:, g : g + 1])

            def load_wk(g):
                wkg = wks.tile([128, D_MODEL], BF16, tag="wks")
                nc.gpsimd.dma_start(wkg[:], wk[g])
                return wkg

            def load_wq(g):
                wqg = wqs.tile([128, D_MODEL], BF16, tag="wqs")
                nc.gpsimd.dma_start(wqg[:], wq[g])
                return wqg

            def proj_unit_gen():
                # pairs g>=1, deferred into the attention loop
                for g in range(1, NPAIR):
                    wkg = load_wk(g)
                    yield emit_kproj, (0, g, wkg)
                    yield emit_kproj, (1, g, wkg)
                    wqg = load_wq(g)
                    yield emit_qproj, (g, wqg)
                kq_es.close()

            with (
                P(name="expp", bufs=6) as expp,
                P(name="psS", bufs=2, space="PSUM") as psS,
                P(name="psacc", bufs=1, space="PSUM") as psacc,
                P(name="psP", bufs=1, space="PSUM") as psP,
                P(name="sgx", bufs=2) as sgxp,
                P(name="pxg", bufs=1) as pxg,
            ):
                # kq pools opened inside the attention pool scope so they
                # can be closed (and their SBUF reused) once the last
                # projection has been emitted
                kq_es = contextlib.ExitStack()
                kstr = kq_es.enter_context(P(name="kstr", bufs=1))
                qstr = kq_es.enter_context(P(name="qstr", bufs=1))
                wks = kq_es.enter_context(P(name="wks", bufs=2))
                wqs = kq_es.enter_context(P(name="wqs", bufs=2))

                kt_sb = kstr.tile([128, NK * S], BF16, tag="ktsb")
                nc.gpsimd.dma_start(kt_sb[:], kt[:])
                qt_sb = qstr.tile([128, NK * QL], BF16, tag="qtsb")
                nc.scalar.dma_start(qt_sb[:], qt[:])

                # pair 0 projections upfront
                wkg0 = load_wk(0)
                emit_kproj(0, 0, wkg0)
                emit_kproj(1, 0, wkg0)
                wqg0 = load_wq(0)
                emit_qproj(0, wqg0)

                proj_iter = proj_unit_gen()

                def emit_scores(g, qb, cg):
                    ktg, qtg = KT[g], QT[g]
                    q0 = 512 * qb
                    tiles = [
                        psS.tile([128, QL], F32, name=f"sc{h}", tag="scores")
                        for h in range(2)
                    ]
                    # alternate row groups (h) so each LDW overlaps the
                    # other head's stream
                    for ci in range(2):
                        c = 2 * cg + ci
                        for h in range(2):
                            p0 = 64 * h
                            nc.tensor.matmul(
                                tiles[h][:, 512 * ci : 512 * ci + 512],
                                ktg[p0 : p0 + 64, 128 * c : 128 * c + 128],
                                qtg[p0 : p0 + 64, q0 : q0 + 512],
                                start=True,
                                stop=True,
                                skip_group_check=True,
                            )
                    return tiles

                def emit_spill(g, qb, acc):
                    # X rows -> xg (bf16, SBUF); recip of sum row -> srs stage
                    for h in range(2):
                        if h == 0:
                            # partitions line up: copy straight into xg
                            nc.vector.tensor_copy(
                                xg_tiles[g][0:64, 512 * qb : 512 * qb + 512],
                                acc[h][0:64, :],
                            )
                        else:
                            sx = sgxp.tile([64, 512], BF16, tag="sgx")
                            nc.vector.tensor_copy(sx[:], acc[h][0:64, :])
                            nc.sync.dma_start(
                                xg_tiles[g][64:128, 512 * qb : 512 * qb + 512], sx[:]
                            )
                        sd = sgxp.tile([65, 512], F32, tag="sgd")
                        nc.vector.tensor_copy(sd[64:65, :], acc[h][64:65, :])
                        nc.sync.dma_start(
                            den[g][2 * h + qb : 2 * h + qb + 1, :], sd[64:65, :]
                        )

                def emit_chain(g):
                    # denr = 1/den ; rep = sel.T @ denr (fp16 broadcast MM)
                    with nc.allow_low_precision(reason="fp16 softmax recips"):
                        nc.vector.reciprocal(denr[g][:], den[g][:])
                    rep = psP.tile([128, QL], F32, tag="kproj")
                    for qb in range(2):
                        nc.tensor.matmul(
                            rep[:, 512 * qb : 512 * qb + 512],
                            sel[qb][:],
                            denr[g][:],
                            start=True,
                            stop=True,
                            skip_group_check=True,
                        )
                    xn = pkq.tile([128, QL], BF16, name=f"xn{g}", tag=f"xn{g}")
                    nc.vector.tensor_tensor(xn[:], xg_tiles[g][:], rep[:], MULT)
                    xn_tiles[g] = xn

                pending_spill = None
                pending_chain = None
                slot = 0
                for g in range(NPAIR):
                    xg_tiles[g] = pxg.tile(
                        [128, QL], BF16, name=f"xg{g}", tag=f"xg{g}"
                    )
                    den[g] = denp.tile([4, 512], F32, name=f"den{g}", tag="den")
                    denr[g] = denp.tile([4, 512], F16, name=f"denr{g}", tag="denr")
                    for qb in range(2):
                        acc = [
                            psacc.tile([65, 512], F32, name="acca", tag="acca"),
                            psacc.tile([65, 512], F32, name="accb", tag="accb"),
                        ]
                        sc_cur = emit_scores(g, qb, 0)
                        if pending_spill is not None:
                            emit_spill(*pending_spill)
                            pending_spill = None
                            if g >= 1 and qb == 1:
                                # spills of (g, qb=0,1) for pair g-? -> chain
                                pending_chain = g - 1
                        for cg in range(NT // 2):
                            sc_next = (
                                emit_scores(g, qb, cg + 1)
                                if cg + 1 < NT // 2
                                else None
                            )
                            for h in range(2):
                                hh = 2 * g + h
                                if ATTNV_FP8:
                                    # exp(s/8 - 3): shift keeps ex under the
                                    # fp8e4 max (+-240); cancels in X/d
                                    ex = expp.tile([128, QL], FP8, tag="exp")
                                    with nc.allow_low_precision(
                                        reason="fp8 softmax weights"
                                    ):
                                        nc.scalar.activation(
                                            ex[:],
                                            sc_cur[h][:],
                                            AF.Exp,
                                            scale=0.125,
                                            bias=expb[:],
                                        )
                                    nc.tensor.matmul(
                                        acc[h][:],
                                        vp_tiles[cg][:, :].rearrange(
                                            "p (c w) -> p c w", c=2
                                        )[:, :, 65 * hh : 65 * hh + 65],
                                        ex[:, :].rearrange(
                                            "p (c q) -> p c q", c=2
                                        ),
                                        start=(cg == 0),
                                        stop=(cg == NT // 2 - 1),
                                        perf_mode=mybir.MatmulPerfMode.DoubleRow,
                                        skip_group_check=True,
                                    )
                                else:
                                    ex = expp.tile([128, QL], BF16, tag="exp")
                                    nc.scalar.activation(
                                        ex[:], sc_cur[h][:], AF.Exp, scale=0.125
                                    )
                                    for ci in range(2):
                                        c = 2 * cg + ci
                                        for r0, r1 in _rhs():
                                            nc.tensor.matmul(
                                                acc[h][:],
                                                v_tiles[c][
                                                    r0:r1,
                                                    65 * hh : 65 * hh + 65,
                                                ],
                                                ex[
                                                    r0:r1,
                                                    512 * ci : 512 * ci + 512,
                                                ],
                                                start=(c == 0 and r0 == 0),
                                                stop=(c == NT - 1 and r1 == 128),
                                                skip_group_check=True,
                                            )
                            if slot % 4 == 1:
                                step = next(proj_iter, None)
                                if step is not None:
                                    step[0](*step[1])
                            if pending_chain is not None and cg == 2:
                                emit_chain(pending_chain)
                                pending_chain = None
                            slot += 1
                            sc_cur = sc_next
                        pending_spill = (g, qb, acc)
                emit_spill(*pending_spill)
                for step in proj_iter:
                    step[0](*step[1])
                emit_chain(7)

            # ---- output projection ---------------------------------------
            with (
                P(name="pwo", bufs=1) as pwo,
                P(name="ps3o", bufs=4, space="PSUM") as ps3o,
            ):
            bor_t = consts.tile([128, D_MODEL], BF16, tag="bor")
            nc.scalar.dma_start(bor_t[:], bor[:])
            wo_sb = pwo.tile([128, NPAIR * D_MODEL], BF16, tag="wosb")
            nc.scalar.dma_start(wo_sb[:], wo[:])

            qrr = 0
            for m in range(QL // 128):
                for j in range(2):
                    ps = ps3o.tile([128, 512], F32, tag="oproj")
                    for g in range(NPAIR):
                        for r0, r1 in _rhs():
                            nc.tensor.matmul(
                                ps[:],
                                xn_tiles[g][
                                    r0:r1, 128 * m : 128 * m + 128
                                ],
                                wo_sb[
                                    r0:r1,
                                    1024 * g + 512 * j : 1024 * g + 512 * j + 512,
                                ],
                                start=(g == 0 and r0 == 0),
                                stop=(g == NPAIR - 1 and r1 == 128),
                                skip_group_check=True,
                            )
                    ot = stg.tile([128, 512], F32, tag="outs")
                    nc.vector.tensor_tensor(
                        ot[:], ps[:], bor_t[:, 512 * j : 512 * j + 512], ADD
                    )
                    eng = (nc.sync, nc.scalar, nc.gpsimd)[qrr % 3]
                    qrr += 1
                    eng.dma_start(
                        out[128 * m : 128 * m + 128, 512 * j : 512 * j + 512], ot[:]
                    )


_NC_CACHE = None
LAST_RESULT = None


def _get_nc():
    global _NC_CACHE
    if _NC_CACHE is None:
        _install_patch()
        _NC_CACHE = _build_bass()
    return _NC_CACHE


def kernel(q, k, v, w_q, b_q, w_k, b_k, w_v, b_v, w_o, b_o):
    global LAST_RESULT
    import ml_dtypes

    q = np.asarray(q, np.float32)
    k = np.asarray(k, np.float32)
    v = np.asarray(v, np.float32)

    def _pair_w(w):
        # [in, out] -> [g, 128, 1024]: [g][p, 128k+j] = w[128k+p, 128g+j]
        return np.ascontiguousarray(
            np.asarray(w, np.float32)
            .reshape(NK, 128, NPAIR, 128)
            .transpose(2, 1, 0, 3)
            .reshape(NPAIR, 128, D_MODEL)
        ).astype(ml_dtypes.bfloat16)

    def _chunk_w(w):
        # [in, out] -> [128, 8*1024]: [p, 1024k+o] = w[128k+p, o]
        return np.ascontiguousarray(
            np.asarray(w, np.float32)
            .reshape(NK, 128, D_MODEL)
            .transpose(1, 0, 2)
            .reshape(128, NK * D_MODEL)
        ).astype(ml_dtypes.bfloat16)

    w_q = _pair_w(w_q)
    w_k = _pair_w(w_k)
    w_v = _chunk_w(w_v)
    # wo: [p, 1024g+o] = w_o[128g+p, o] -- same transform (g indexes chunks)
    w_o = _chunk_w(w_o)
    b_q = np.asarray(b_q, np.float32)
    b_k = np.asarray(b_k, np.float32)
    b_v = np.asarray(b_v, np.float32)
    b_o = np.asarray(b_o, np.float32)

    bqt = np.ascontiguousarray(b_q.reshape(NK, 128).T)
    bkt = np.ascontiguousarray(b_k.reshape(NK, 128).T)
    bvr = np.ascontiguousarray(
        np.broadcast_to(b_v[None, :], (128, D_MODEL))
    ).astype(ml_dtypes.bfloat16)
    bor = np.ascontiguousarray(
        np.broadcast_to(b_o[None, :], (128, D_MODEL))
    ).astype(ml_dtypes.bfloat16)
    selq = np.zeros((2, 4, 128), np.float16)
    for qb in range(2):
        selq[qb, qb, 0:64] = 1.0
        selq[qb, 2 + qb, 64:128] = 1.0

    in_maps = []
    for c in range(N_CORES):
        b = c // 2
        r0 = QL * (c % 2)
        # qt: [p, 1024k+t] = q_proj_input^T chunked
        qtc = np.ascontiguousarray(
            q[b, r0 : r0 + QL, :].T.reshape(NK, 128, QL).transpose(1, 0, 2).reshape(
                128, NK * QL
            )
        ).astype(ml_dtypes.bfloat16)
        ktc = np.ascontiguousarray(
            k[b].T.reshape(NK, 128, S).transpose(1, 0, 2).reshape(128, NK * S)
        ).astype(ml_dtypes.bfloat16)
        in_maps.append(
            {
                "qt": qtc,
                "kt": ktc,
                "vt": np.ascontiguousarray(
                    v[b]
                    .T.reshape(8, 128, 16, 128)
                    .transpose(2, 1, 0, 3)
                    .reshape(16, 128, 1024)
                ).astype(ml_dtypes.bfloat16),
                "wq": w_q,
                "wk": w_k,
                "wv": w_v,
                "wo": w_o,
                "bqt": bqt,
                "bkt": bkt,
                "bvr": bvr,
                "bor": bor,
                "selq": selq,
            }
        )

    nc = _get_nc()
    res = run_bass_kernel_spmd(nc, in_maps, list(range(N_CORES)))
    LAST_RESULT = res

    outp = np.empty((B, S, D_MODEL), np.float32)
    for c in range(N_CORES):
        b = c // 2
        r0 = QL * (c % 2)
        outp[b, r0 : r0 + QL, :] = np.asarray(res.results[c]["out"], np.float32)
    return outp


# revision 29
# speedup vs baseline: 1.2293x; 1.0052x over previous
"""Multi-head attention (B=4, S=2048, d_model=1024, H=16) on 8 TRN2 NeuronCores.

Sharding: core c handles batch c//2 and query rows [1024*(c%2), +1024).
Each core redundantly projects K/V for its batch (no collectives) and
produces a disjoint [1024, 1024] slice of the output.

v2 structure (vs baseline):
  - every K=128 matmul is split into two K=64 row-group matmuls emitted
    alternately, so each LDWEIGHTS targets the row half not currently
    streaming (PE pulls it ahead; halves can also run concurrently).
  - input DMAs use 2KB+ rows and are spread across sync/scalar/vector/
    gpsimd queues; ones-columns of V are memset, not DMAed.
  - unnormalized attention output stays in SBUF (bf16); softmax sums go
    to tiny per-pair den tiles; recip broadcast via fp16 selector matmul.
  - projections for g>=1 interleave into the attention loop from g=0;
    per-pair chains run as soon as a pair's spills land.
"""

import numpy as np

import bass_rust
import concourse.bass as bass
import concourse.mybir as mybir
import concourse.tile as tile
from concourse.bass_utils import run_bass_kernel_spmd
from concourse.vector_clock import ScopedClock

F32 = mybir.dt.float32
F16 = mybir.dt.float16
FP8 = mybir.dt.float8e4
BF16 = mybir.dt.bfloat16
AF = mybir.ActivationFunctionType
ADD = mybir.AluOpType.add
MULT = mybir.AluOpType.mult

D_MODEL = 1024
B = 4
S = 2048
N_CORES = 8
QL = 1024  # query rows per core
NPAIR = 8  # head pairs
NK = D_MODEL // 128  # contraction chunks
NT = S // 128  # key chunks
VPW = 65 * 16  # padded V width
ROW_SPLIT = False  # split K=128 matmuls into two K=64 row-group matmuls
ATTNV_FP8 = False  # attnV via fp8e4 DoubleRow (chunk-pairs, exp shifted by -3)


def _rhs():
    return ((0, 64), (64, 128)) if ROW_SPLIT else ((0, 128),)

# ---------------------------------------------------------------------------
# Workaround for this container's walrus build: each instruction may carry at
# most ONE embedded sync-wait ("Too many sync wait commands" otherwise). Tile
# attaches several; split the extras onto same-engine NOPs placed immediately
# before the instruction (engine queues are in-order => identical semantics).
_MAX_WAITS = 1


def _patched_lower(self, ordered):
    nc = self.nc
    for bb_name, insts in ordered.items():
        new_list = []
        for inst in insts:
            si = inst.sync_info
            waits = list(si.on_wait) if si is not None and si.on_wait else []
            if len(waits) > _MAX_WAITS:
                updates = list(si.on_update) if si.on_update else []
                for w in waits[:-_MAX_WAITS]:
                    nop = bass_rust.InstNoOp(
                        name=nc.get_next_instruction_name(),
                        engine=inst.engine,
                        debug=inst.debug,
                        sync_info=bass_rust.SyncInfo(on_wait=[w], on_update=[]),
                    )
                    new_list.append(nop)
                inst.sync_info = bass_rust.SyncInfo(
                    on_wait=waits[-_MAX_WAITS:], on_update=updates
                )
            new_list.append(inst)
        insts[:] = new_list
    return tile.TileContext._orig_lower_ordered_insts(self, ordered)


def _patched_drain(self, tick_clock, wait_clock):
    probe = self.nc.sync.nop(nofuse=True)
    wait_clock.add_sem_waits(probe.ins, ScopedClock({None: tick_clock.global_clock}))
    si = probe.ins.sync_info
    waits = list(si.on_wait) if si is not None and si.on_wait else []
    if len(waits) > _MAX_WAITS:
        probe.ins.sync_info = bass_rust.SyncInfo(
            on_wait=waits[:_MAX_WAITS], on_update=[]
        )
        for w in waits[_MAX_WAITS:]:
            n = self.nc.sync.nop(nofuse=True)
            n.ins.sync_info = bass_rust.SyncInfo(on_wait=[w], on_update=[])
    self.nc.sync.drain()
    self.nc.all_engine_barrier()
    assert self.sems is not None
    popped = self.nc._tile_sem_poison_stack.pop()
    assert popped is self._sem_poison
    self.nc.clear_and_free_semaphores(list(self.sems.allocated().values()))
    self.nc.all_engine_barrier()


import concourse.bass_utils as _bu


def _install_patch():
    if not hasattr(tile.TileContext, "_orig_lower_ordered_insts"):
        tile.TileContext._orig_lower_ordered_insts = (
            tile.TileContext._lower_ordered_insts
        )
        tile.TileContext._lower_ordered_insts = _patched_lower
        tile.TileContext._drain_and_barrier = _patched_drain


# ---------------------------------------------------------------------------


def _build_bass():
    nc = bass.Bass()
    qt = nc.dram_tensor("qt", [128, NK * QL], BF16, kind="ExternalInput")
    kt = nc.dram_tensor("kt", [128, NK * S], BF16, kind="ExternalInput")
    vt = nc.dram_tensor("vt", [NT, 128, 1024], BF16, kind="ExternalInput")
    wq = nc.dram_tensor("wq", [NPAIR, 128, D_MODEL], BF16, kind="ExternalInput")
    wk = nc.dram_tensor("wk", [NPAIR, 128, D_MODEL], BF16, kind="ExternalInput")
    wv = nc.dram_tensor("wv", [128, NK * D_MODEL], BF16, kind="ExternalInput")
    wo = nc.dram_tensor("wo", [128, NPAIR * D_MODEL], BF16, kind="ExternalInput")
    bqt = nc.dram_tensor("bqt", [128, NK], F32, kind="ExternalInput")
    bkt = nc.dram_tensor("bkt", [128, NK], F32, kind="ExternalInput")
    bvr = nc.dram_tensor("bvr", [128, D_MODEL], BF16, kind="ExternalInput")
    bor = nc.dram_tensor("bor", [128, D_MODEL], BF16, kind="ExternalInput")
    selq = nc.dram_tensor("selq", [2, 4, 128], F16, kind="ExternalInput")
    out = nc.dram_tensor("out", [QL, D_MODEL], F32, kind="ExternalOutput")

    with tile.TileContext(nc) as tc:
        _emit(nc, tc, locals())
    return nc


def _emit(nc, tc, t):
    qt, kt, vt = t["qt"], t["kt"], t["vt"]
    wq, wk, wv, wo = t["wq"], t["wk"], t["wv"], t["wo"]
    bqt, bkt, bvr, bor = t["bqt"], t["bkt"], t["bvr"], t["bor"]
    selq, out = t["selq"], t["out"]

    P = tc.tile_pool

    with (
        P(name="consts", bufs=1) as consts,
        P(name="stg", bufs=3) as stg,
        P(name="sel", bufs=1) as selp,
        P(name="den", bufs=3) as denp,
    ):
        bqt_t = consts.tile([128, NK], F32, tag="bqt")
        nc.gpsimd.dma_start(bqt_t[:], bqt[:])
        bkt_t = consts.tile([128, NK], F32, tag="bkt")
        nc.gpsimd.dma_start(bkt_t[:], bkt[:])
        bvr_t = consts.tile([128, D_MODEL], BF16, tag="bvr")
        nc.gpsimd.dma_start(bvr_t[:], bvr[:])

        # selector tiles for the recip broadcast: sel[qb][r, p] = 1 iff
        # r == 2*(p//64) + qb  (fp16, host constant)
        sel = []
        for qb in range(2):
            st = selp.tile([4, 128], F16, name=f"sel{qb}", tag=f"sel{qb}")
            nc.gpsimd.dma_start(st[:], selq[qb])
            sel.append(st)
        expb = selp.tile([128, 1], F32, tag="expb")
        nc.gpsimd.memset(expb[:], -3.0)
        # per-pair softmax-denominator tiles [4, 512] (row r = 2*h + qb)
        den = [None] * NPAIR
        denr = [None] * NPAIR

        with P(name="pv", bufs=1) as pv, P(name="pkq", bufs=1) as pkq:
            # ---- V projection: V_pad [t, 16*65] row-major, bf16 ----------
            VDT = FP8 if ATTNV_FP8 else BF16
            if ATTNV_FP8:
                # chunk-PAIR tiles [128, 2*VPW] for DoubleRow ([Ki, Ko=2, M])
                vp_tiles = [
                    pv.tile([128, 2 * VPW], FP8, name=f"v{cp}", tag=f"v{cp}")
                    for cp in range(NT // 2)
                ]
                def vslice(c):
                    return vp_tiles[c // 2][
                        :, VPW * (c % 2) : VPW * (c % 2) + VPW
                    ]
            else:
                v_tiles = [
                    pv.tile([128, VPW], BF16, name=f"v{c}", tag=f"v{c}")
                    for c in range(NT)
                ]
                def vslice(c):
                    return v_tiles[c][:, :]
            # ones columns (col 65h+64) provide softmax sums in attnV
            for c in range(NT):
                nc.gpsimd.memset(
                    vslice(c).rearrange("p (h w) -> p h w", w=65)[:, :, 64:65],
                    1.0,
                )

            with (
                P(name="wvp", bufs=1) as wvp,
                P(name="vstr", bufs=5) as vstr,
                P(name="psV", bufs=3, space="PSUM") as psV,
            ):
                wv_sb = wvp.tile([128, NK * D_MODEL], BF16, tag="wvsb")
                nc.scalar.dma_start(wv_sb[:, 0:4096], wv[:, 0:4096])
                nc.gpsimd.dma_start(wv_sb[:, 4096:8192], wv[:, 4096:8192])
                for c in range(NT):
                    vts = vstr.tile([128, 1024], BF16, tag="vts")
                    nc.sync.dma_start(vts[:], vt[c])
                    ps = psV.tile([128, 1024], F32, tag="vproj")
                    for k in range(NK):
                        for j in range(2):
                            for r0, r1 in _rhs():
                                nc.tensor.matmul(
                                    ps[:, 512 * j : 512 * j + 512],
                                    vts[r0:r1, 128 * k : 128 * k + 128],
                                    wv_sb[
                                        r0:r1,
                                        1024 * k + 512 * j : 1024 * k + 512 * j + 512,
                                    ],
                                    start=(k == 0 and r0 == 0),
                                    stop=(k == NK - 1 and r1 == 128),
                                    skip_group_check=True,
                                )
                    dst = vslice(c).rearrange("p (h w) -> p h w", w=65)[:, :, 0:64]
                    with nc.allow_low_precision(
                        reason="fp8 V for DoubleRow attnV; softmax-averaged"
                    ):
                        nc.vector.tensor_tensor(
                            dst,
                            ps[:, :].rearrange("p (h w) -> p h w", w=64),
                            bvr_t[:, :].rearrange("p (h w) -> p h w", w=64),
                            ADD,
                        )

            # ---- K/Q projections interleaved into attention --------------
            xn_tiles = [None] * NPAIR
            xg_tiles = [None] * NPAIR
            KT = [
                pkq.tile([128, S], BF16, name=f"ktg{g}", tag=f"ktg{g}")
                for g in range(NPAIR)
            ]
            QT = [
                pkq.tile([128, QL], BF16, name=f"qtg{g}", tag=f"qtg{g}")
                for g in range(NPAIR)
            ]

            import contextlib

            def emit_kproj(half, g, wkg):
                # KT[g][:, 1024*half:+1024] = (wk[g].T @ K^T)(half) + bias
                ps = psP.tile([128, 1024], F32, tag="kproj")
                for k in range(NK):
                    for j in range(2):
                        for r0, r1 in _rhs():
                            nc.tensor.matmul(
                                ps[:, 512 * j : 512 * j + 512],
                                wkg[r0:r1, 128 * k : 128 * k + 128],
                                kt_sb[
                                    r0:r1,
                                    2048 * k
                                    + 1024 * half
                                    + 512 * j : 2048 * k
                                    + 1024 * half
                                    + 512 * j
                                    + 512,
                                ],
                                start=(k == 0 and r0 == 0),
                                stop=(k == NK - 1 and r1 == 128),
                                skip_group_check=True,
                            )
                nc.vector.tensor_scalar_add(
                    KT[g][:, 1024 * half : 1024 * half + 1024],
                    ps[:],
                    bkt_t[:, g : g + 1],
                )

            def emit_qproj(g, wqg):
                ps = psP.tile([128, 1024], F32, tag="kproj")
                for k in range(NK):
                    for j in range(2):
                        for r0, r1 in _rhs():
                            nc.tensor.matmul(
                                ps[:, 512 * j : 512 * j + 512],
                                wqg[r0:r1, 128 * k : 128 * k + 128],
                                qt_sb[
                                    r0:r1,
                                    1024 * k + 512 * j : 1024 * k + 512 * j + 512,
                                ],
                                start=(k == 0 and r0 == 0),
                                stop=(k == NK - 1 and r1 == 128),
                                skip_group_check=True,
                            )
                nc.vector.tensor_scalar_add(QT[g][:], ps[:], bqt_t[:, g : g + 1])

            def load_wk(g):
                wkg = wks.tile([128, D_MODEL], BF16, tag="wks")
                nc.gpsimd.dma_start(wkg[:], wk[g])
                return wkg

            def load_wq(g):
                wqg = wqs.tile([128, D_MODEL], BF16, tag="wqs")
                nc.gpsimd.dma_start(wqg[:], wq[g])
                return wqg

            def proj_unit_gen():
                # pairs g>=1, deferred into the attention loop
                for g in range(1, NPAIR):
                    wkg = load_wk(g)
                    yield emit_kproj, (0, g, wkg)
                    yield emit_kproj, (1, g, wkg)
                    wqg = load_wq(g)
                    yield emit_qproj, (g, wqg)
                kq_es.close()

            with (
                P(name="expp", bufs=6) as expp,
                P(name="psS", bufs=2, space="PSUM") as psS,
                P(name="psacc", bufs=1, space="PSUM") as psacc,
                P(name="psP", bufs=1, space="PSUM") as psP,
                P(name="sgx", bufs=2) as sgxp,
                P(name="pxg", bufs=1) as pxg,
            ):
                # kq pools opened inside the attention pool scope so they
                # can be closed (and their SBUF reused) once the last
                # projection has been emitted
                kq_es = contextlib.ExitStack()
                kstr = kq_es.enter_context(P(name="kstr", bufs=1))
                qstr = kq_es.enter_context(P(name="qstr", bufs=1))
                wks = kq_es.enter_context(P(name="wks", bufs=2))
                wqs = kq_es.enter_context(P(name="wqs", bufs=2))

                kt_sb = kstr.tile([128, NK * S], BF16, tag="ktsb")
                nc.gpsimd.dma_start(kt_sb[:], kt[:])
                qt_sb = qstr.tile([128, NK * QL], BF16, tag="qtsb")
                nc.scalar.dma_start(qt_sb[:], qt[:])

                # pair 0 projections upfront
                wkg0 = load_wk(0)
                emit_kproj(0, 0, wkg0)
                emit_kproj(1, 0, wkg0)
                wqg0 = load_wq(0)
                emit_qproj(0, wqg0)

                proj_iter = proj_unit_gen()

                def emit_scores(g, qb, cg):
                    ktg, qtg = KT[g], QT[g]
                    q0 = 512 * qb
                    tiles = [
                        psS.tile([128, QL], F32, name=f"sc{h}", tag="scores")
                        for h in range(2)
                    ]
                    # alternate row groups (h) so each LDW overlaps the
                    # other head's stream
                    for ci in range(2):
                        c = 2 * cg + ci
                        for h in range(2):
                            p0 = 64 * h
                            nc.tensor.matmul(
                                tiles[h][:, 512 * ci : 512 * ci + 512],
                                ktg[p0 : p0 + 64, 128 * c : 128 * c + 128],
                                qtg[p0 : p0 + 64, q0 : q0 + 512],
                                start=True,
                                stop=True,
                                skip_group_check=True,
                            )
                    return tiles

                def emit_spill(g, qb, acc):
                    # X rows -> xg (bf16, SBUF); recip of sum row -> srs stage
                    for h in range(2):
                        if h == 0:
                            # partitions line up: copy straight into xg
                            nc.vector.tensor_copy(
                                xg_tiles[g][0:64, 512 * qb : 512 * qb + 512],
                                acc[h][0:64, :],
                            )
                        else:
                            sx = sgxp.tile([64, 512], BF16, tag="sgx")
                            nc.vector.tensor_copy(sx[:], acc[h][0:64, :])
                            nc.sync.dma_start(
                                xg_tiles[g][64:128, 512 * qb : 512 * qb + 512], sx[:]
                            )
                        sd = sgxp.tile([65, 512], F32, tag="sgd")
                        nc.vector.tensor_copy(sd[64:65, :], acc[h][64:65, :])
                        nc.sync.dma_start(
                            den[g][2 * h + qb : 2 * h + qb + 1, :], sd[64:65, :]
                        )

                def emit_chain(g):
                    # denr = 1/den ; rep = sel.T @ denr (fp16 broadcast MM)
                    with nc.allow_low_precision(reason="fp16 softmax recips"):
                        nc.vector.reciprocal(denr[g][:], den[g][:])
                    rep = psP.tile([128, QL], F32, tag="kproj")
                    for qb in range(2):
                        nc.tensor.matmul(
                            rep[:, 512 * qb : 512 * qb + 512],
                            sel[qb][:],
                            denr[g][:],
                            start=True,
                            stop=True,
                            skip_group_check=True,
                        )
                    xn = pkq.tile([128, QL], BF16, name=f"xn{g}", tag=f"xn{g}")
                    nc.vector.tensor_tensor(xn[:], xg_tiles[g][:], rep[:], MULT)
                    xn_tiles[g] = xn

                pending_spill = None
                pending_chain = None
                slot = 0
                for g in range(NPAIR):
                    xg_tiles[g] = pxg.tile(
                        [128, QL], BF16, name=f"xg{g}", tag=f"xg{g}"
                    )
                    den[g] = denp.tile([4, 512], F32, name=f"den{g}", tag="den")
                    denr[g] = denp.tile([4, 512], F16, name=f"denr{g}", tag="denr")
                    for qb in range(2):
                        acc = [
                            psacc.tile([65, 512], F32, name="acca", tag="acca"),
                            psacc.tile([65, 512], F32, name="accb", tag="accb"),
                        ]
                        sc_cur = emit_scores(g, qb, 0)
                        if pending_spill is not None:
                            emit_spill(*pending_spill)
                            pending_spill = None
                            if g >= 1 and qb == 1:
                                # spills of (g, qb=0,1) for pair g-? -> chain
                                pending_chain = g - 1
                        for cg in range(NT // 2):
                            sc_next = (
                                emit_scores(g, qb, cg + 1)
                                if cg + 1 < NT // 2
                                else None
                            )
                            for h in range(2):
                                hh = 2 * g + h
                                if ATTNV_FP8:
                                    # exp(s/8 - 3): shift keeps ex under the
                                    # fp8e4 max (+-240); cancels in X/d
                                    ex = expp.tile([128, QL], FP8, tag="exp")
                                    with nc.allow_low_precision(
                                        reason="fp8 softmax weights"
                                    ):
                                        nc.scalar.activation(
                                            ex[:],
                                            sc_cur[h][:],
                                            AF.Exp,
                                            scale=0.125,
                                            bias=expb[:],
                                        )
                                    nc.tensor.matmul(
                                        acc[h][:],
                                        vp_tiles[cg][:, :].rearrange(
                                            "p (c w) -> p c w", c=2
                                        )[:, :, 65 * hh : 65 * hh + 65],
                                        ex[:, :].rearrange(
                                            "p (c q) -> p c q", c=2
                                        ),
                                        start=(cg == 0),
                                        stop=(cg == NT // 2 - 1),
                                        perf_mode=mybir.MatmulPerfMode.DoubleRow,
                                        skip_group_check=True,
                                    )
                                else:
                                    ex = expp.tile([128, QL], BF16, tag="exp")
                                    nc.scalar.activation(
                                        ex[:], sc_cur[h][:], AF.Exp, scale=0.125
                                    )
                                    for ci in range(2):
                                        c = 2 * cg + ci
                                        for r0, r1 in _rhs():
                                            nc.tensor.matmul(
                                                acc[h][:],
                                                v_tiles[c][
                                                    r0:r1,
                                                    65 * hh : 65 * hh + 65,
                                                ],
                                                ex[
                                                    r0:r1,
                                                    512 * ci : 512 * ci + 512,
                                                ],
                                                start=(c == 0 and r0 == 0),
                                                stop=(c == NT - 1 and r1 == 128),
                                                skip_group_check=True,
                                            )
                            if slot % 4 == 1:
                                step = next(proj_iter, None)
                                if step is not None:
                                    step[0](*step[1])
                            if pending_chain is not None and cg == 2:
                                emit_chain(pending_chain)
                                pending_chain = None
                            slot += 1
                            sc_cur = sc_next
                        pending_spill = (g, qb, acc)
                emit_spill(*pending_spill)
                for step in proj_iter:
                    step[0](*step[1])
                emit_chain(7)

            # ---- output projection ---------------------------------------
            with (
                P(name="pwo", bufs=1) as pwo,
                P(name="ps3o", bufs=4, space="PSUM") as ps3o,
            ):
            bor_t = consts.tile([128, D_MODEL], BF16, tag="bor")
            nc.scalar.dma_start(bor_t[:], bor[:])
            wo_sb = pwo.tile([128, NPAIR * D_MODEL], BF16, tag="wosb")
            nc.scalar.dma_start(wo_sb[:], wo[:])

            qrr = 0
            for m in range(QL // 128):
                for j in range(2):
                    ps = ps3o.tile([128, 512], F32, tag="oproj")
                    for g in range(NPAIR):
                        for r0, r1 in _rhs():
                            nc.tensor.matmul(
                                ps[:],
                                xn_tiles[g][
                                    r0:r1, 128 * m : 128 * m + 128
                                ],
                                wo_sb[
                                    r0:r1,
                                    1024 * g + 512 * j : 1024 * g + 512 * j + 512,
                                ],
                                start=(g == 0 and r0 == 0),
                                stop=(g == NPAIR - 1 and r1 == 128),
                                skip_group_check=True,
                            )
                    ot = stg.tile([128, 512], F32, tag="outs")
                    nc.vector.tensor_tensor(
                        ot[:], ps[:], bor_t[:, 512 * j : 512 * j + 512], ADD
                    )
                    eng = (nc.sync, nc.scalar, nc.gpsimd)[qrr % 3]
                    qrr += 1
                    eng.dma_start(
                        out[128 * m : 128 * m + 128, 512 * j : 512 * j + 512], ot[:]
                    )


_NC_CACHE = None
LAST_RESULT = None


def _get_nc():
    global _NC_CACHE
    if _NC_CACHE is None:
        _install_patch()
        _NC_CACHE = _build_bass()
    return _NC_CACHE


def kernel(q, k, v, w_q, b_q, w_k, b_k, w_v, b_v, w_o, b_o):
    global LAST_RESULT
    import ml_dtypes

    q = np.asarray(q, np.float32)
    k = np.asarray(k, np.float32)
    v = np.asarray(v, np.float32)

    def _pair_w(w):
        # [in, out] -> [g, 128, 1024]: [g][p, 128k+j] = w[128k+p, 128g+j]
        return np.ascontiguousarray(
            np.asarray(w, np.float32)
            .reshape(NK, 128, NPAIR, 128)
            .transpose(2, 1, 0, 3)
            .reshape(NPAIR, 128, D_MODEL)
        ).astype(ml_dtypes.bfloat16)

    def _chunk_w(w):
        # [in, out] -> [128, 8*1024]: [p, 1024k+o] = w[128k+p, o]
        return np.ascontiguousarray(
            np.asarray(w, np.float32)
            .reshape(NK, 128, D_MODEL)
            .transpose(1, 0, 2)
            .reshape(128, NK * D_MODEL)
        ).astype(ml_dtypes.bfloat16)

    w_q = _pair_w(w_q)
    w_k = _pair_w(w_k)
    w_v = _chunk_w(w_v)
    # wo: [p, 1024g+o] = w_o[128g+p, o] -- same transform (g indexes chunks)
    w_o = _chunk_w(w_o)
    b_q = np.asarray(b_q, np.float32)
    b_k = np.asarray(b_k, np.float32)
    b_v = np.asarray(b_v, np.float32)
    b_o = np.asarray(b_o, np.float32)

    bqt = np.ascontiguousarray(b_q.reshape(NK, 128).T)
    bkt = np.ascontiguousarray(b_k.reshape(NK, 128).T)
    bvr = np.ascontiguousarray(
        np.broadcast_to(b_v[None, :], (128, D_MODEL))
    ).astype(ml_dtypes.bfloat16)
    bor = np.ascontiguousarray(
        np.broadcast_to(b_o[None, :], (128, D_MODEL))
    ).astype(ml_dtypes.bfloat16)
    selq = np.zeros((2, 4, 128), np.float16)
    for qb in range(2):
        selq[qb, qb, 0:64] = 1.0
        selq[qb, 2 + qb, 64:128] = 1.0

    in_maps = []
    for c in range(N_CORES):
        b = c // 2
        r0 = QL * (c % 2)
        # qt: [p, 1024k+t] = q_proj_input^T chunked
        qtc = np.ascontiguousarray(
            q[b, r0 : r0 + QL, :].T.reshape(NK, 128, QL).transpose(1, 0, 2).reshape(
                128, NK * QL
            )
        ).astype(ml_dtypes.bfloat16)
        ktc = np.ascontiguousarray(
            k[b].T.reshape(NK, 128, S).transpose(1, 0, 2).reshape(128, NK * S)
        ).astype(ml_dtypes.bfloat16)
        in_maps.append(
            {
                "qt": qtc,
                "kt": ktc,
                "vt": np.ascontiguousarray(
                    v[b]
                    .T.reshape(8, 128, 16, 128)
                    .transpose(2, 1, 0, 3)
                    .reshape(16, 128, 1024)
                ).astype(ml_dtypes.bfloat16),
                "wq": w_q,
                "wk": w_k,
                "wv": w_v,
                "wo": w_o,
                "bqt": bqt,
                "bkt": bkt,
                "bvr": bvr,
                "bor": bor,
                "selq": selq,
            }
        )

    nc = _get_nc()
    res = run_bass_kernel_spmd(nc, in_maps, list(range(N_CORES)))
    LAST_RESULT = res

    outp = np.empty((B, S, D_MODEL), np.float32)
    for c in range(N_CORES):
        b = c // 2
        r0 = QL * (c % 2)
        outp[b, r0 : r0 + QL, :] = res.results[c]["out"]
    return outp


# revision 30
# speedup vs baseline: 1.2296x; 1.0003x over previous
"""Multi-head attention (B=4, S=2048, d_model=1024, H=16) on 8 TRN2 NeuronCores.

Sharding: core c handles batch c//2 and query rows [1024*(c%2), +1024).
Each core redundantly projects K/V for its batch (no collectives) and
produces a disjoint [1024, 1024] slice of the output.

v2 structure (vs baseline):
  - every K=128 matmul is split into two K=64 row-group matmuls emitted
    alternately, so each LDWEIGHTS targets the row half not currently
    streaming (PE pulls it ahead; halves can also run concurrently).
  - input DMAs use 2KB+ rows and are spread across sync/scalar/vector/
    gpsimd queues; ones-columns of V are memset, not DMAed.
  - unnormalized attention output stays in SBUF (bf16); softmax sums go
    to tiny per-pair den tiles; recip broadcast via fp16 selector matmul.
  - projections for g>=1 interleave into the attention loop from g=0;
    per-pair chains run as soon as a pair's spills land.
"""

import numpy as np

import bass_rust
import concourse.bass as bass
import concourse.mybir as mybir
import concourse.tile as tile
from concourse.bass_utils import run_bass_kernel_spmd
from concourse.vector_clock import ScopedClock

F32 = mybir.dt.float32
F16 = mybir.dt.float16
FP8 = mybir.dt.float8e4
BF16 = mybir.dt.bfloat16
AF = mybir.ActivationFunctionType
ADD = mybir.AluOpType.add
MULT = mybir.AluOpType.mult

D_MODEL = 1024
B = 4
S = 2048
N_CORES = 8
QL = 1024  # query rows per core
NPAIR = 8  # head pairs
NK = D_MODEL // 128  # contraction chunks
NT = S // 128  # key chunks
VPW = 65 * 16  # padded V width
ROW_SPLIT = False  # split K=128 matmuls into two K=64 row-group matmuls
ATTNV_FP8 = False  # attnV via fp8e4 DoubleRow (chunk-pairs, exp shifted by -3)


def _rhs():
    return ((0, 64), (64, 128)) if ROW_SPLIT else ((0, 128),)

# ---------------------------------------------------------------------------
# Workaround for this container's walrus build: each instruction may carry at
# most ONE embedded sync-wait ("Too many sync wait commands" otherwise). Tile
# attaches several; split the extras onto same-engine NOPs placed immediately
# before the instruction (engine queues are in-order => identical semantics).
_MAX_WAITS = 1


def _patched_lower(self, ordered):
    nc = self.nc
    for bb_name, insts in ordered.items():
        new_list = []
        for inst in insts:
            si = inst.sync_info
            waits = list(si.on_wait) if si is not None and si.on_wait else []
            if len(waits) > _MAX_WAITS:
                updates = list(si.on_update) if si.on_update else []
                for w in waits[:-_MAX_WAITS]:
                    nop = bass_rust.InstNoOp(
                        name=nc.get_next_instruction_name(),
                        engine=inst.engine,
                        debug=inst.debug,
                        sync_info=bass_rust.SyncInfo(on_wait=[w], on_update=[]),
                    )
                    new_list.append(nop)
                inst.sync_info = bass_rust.SyncInfo(
                    on_wait=waits[-_MAX_WAITS:], on_update=updates
                )
            new_list.append(inst)
        insts[:] = new_list
    return tile.TileContext._orig_lower_ordered_insts(self, ordered)


def _patched_drain(self, tick_clock, wait_clock):
    probe = self.nc.sync.nop(nofuse=True)
    wait_clock.add_sem_waits(probe.ins, ScopedClock({None: tick_clock.global_clock}))
    si = probe.ins.sync_info
    waits = list(si.on_wait) if si is not None and si.on_wait else []
    if len(waits) > _MAX_WAITS:
        probe.ins.sync_info = bass_rust.SyncInfo(
            on_wait=waits[:_MAX_WAITS], on_update=[]
        )
        for w in waits[_MAX_WAITS:]:
            n = self.nc.sync.nop(nofuse=True)
            n.ins.sync_info = bass_rust.SyncInfo(on_wait=[w], on_update=[])
    self.nc.sync.drain()
    self.nc.all_engine_barrier()
    assert self.sems is not None
    popped = self.nc._tile_sem_poison_stack.pop()
    assert popped is self._sem_poison
    self.nc.clear_and_free_semaphores(list(self.sems.allocated().values()))
    self.nc.all_engine_barrier()


import concourse.bass_utils as _bu


def _install_patch():
    if not hasattr(tile.TileContext, "_orig_lower_ordered_insts"):
        tile.TileContext._orig_lower_ordered_insts = (
            tile.TileContext._lower_ordered_insts
        )
        tile.TileContext._lower_ordered_insts = _patched_lower
        tile.TileContext._drain_and_barrier = _patched_drain


# ---------------------------------------------------------------------------


def _build_bass():
    nc = bass.Bass()
    qt = nc.dram_tensor("qt", [128, NK * QL], BF16, kind="ExternalInput")
    kt = nc.dram_tensor("kt", [128, NK * S], BF16, kind="ExternalInput")
    vt = nc.dram_tensor("vt", [NT, 128, 1024], BF16, kind="ExternalInput")
    wq = nc.dram_tensor("wq", [NPAIR, 128, D_MODEL], BF16, kind="ExternalInput")
    wk = nc.dram_tensor("wk", [NPAIR, 128, D_MODEL], BF16, kind="ExternalInput")
    wv = nc.dram_tensor("wv", [128, NK * D_MODEL], BF16, kind="ExternalInput")
    wo = nc.dram_tensor("wo", [128, NPAIR * D_MODEL], BF16, kind="ExternalInput")
    bqt = nc.dram_tensor("bqt", [128, NK], F32, kind="ExternalInput")
    bkt = nc.dram_tensor("bkt", [128, NK], F32, kind="ExternalInput")
    bvr = nc.dram_tensor("bvr", [128, D_MODEL], BF16, kind="ExternalInput")
    bor = nc.dram_tensor("bor", [128, D_MODEL], BF16, kind="ExternalInput")
    selq = nc.dram_tensor("selq", [2, 4, 128], F16, kind="ExternalInput")
    out = nc.dram_tensor("out", [QL, D_MODEL], F32, kind="ExternalOutput")

    with tile.TileContext(nc) as tc:
        _emit(nc, tc, locals())
    return nc


def _emit(nc, tc, t):
    qt, kt, vt = t["qt"], t["kt"], t["vt"]
    wq, wk, wv, wo = t["wq"], t["wk"], t["wv"], t["wo"]
    bqt, bkt, bvr, bor = t["bqt"], t["bkt"], t["bvr"], t["bor"]
    selq, out = t["selq"], t["out"]

    P = tc.tile_pool

    with (
        P(name="consts", bufs=1) as consts,
        P(name="stg", bufs=3) as stg,
        P(name="sel", bufs=1) as selp,
        P(name="den", bufs=3) as denp,
    ):
        bqt_t = consts.tile([128, NK], F32, tag="bqt")
        nc.sync.dma_start(bqt_t[:], bqt[:])
        bkt_t = consts.tile([128, NK], F32, tag="bkt")
        nc.sync.dma_start(bkt_t[:], bkt[:])
        bvr_t = consts.tile([128, D_MODEL], BF16, tag="bvr")
        nc.sync.dma_start(bvr_t[:], bvr[:])

        # selector tiles for the recip broadcast: sel[qb][r, p] = 1 iff
        # r == 2*(p//64) + qb  (fp16, host constant)
        sel = []
        for qb in range(2):
            st = selp.tile([4, 128], F16, name=f"sel{qb}", tag=f"sel{qb}")
            nc.sync.dma_start(st[:], selq[qb])
            sel.append(st)
        expb = selp.tile([128, 1], F32, tag="expb")
        nc.gpsimd.memset(expb[:], -3.0)
        # per-pair softmax-denominator tiles [4, 512] (row r = 2*h + qb)
        den = [None] * NPAIR
        denr = [None] * NPAIR

        with P(name="pv", bufs=1) as pv, P(name="pkq", bufs=1) as pkq:
            # ---- V projection: V_pad [t, 16*65] row-major, bf16 ----------
            VDT = FP8 if ATTNV_FP8 else BF16
            if ATTNV_FP8:
                # chunk-PAIR tiles [128, 2*VPW] for DoubleRow ([Ki, Ko=2, M])
                vp_tiles = [
                    pv.tile([128, 2 * VPW], FP8, name=f"v{cp}", tag=f"v{cp}")
                    for cp in range(NT // 2)
                ]
                def vslice(c):
                    return vp_tiles[c // 2][
                        :, VPW * (c % 2) : VPW * (c % 2) + VPW
                    ]
            else:
                v_tiles = [
                    pv.tile([128, VPW], BF16, name=f"v{c}", tag=f"v{c}")
                    for c in range(NT)
                ]
                def vslice(c):
                    return v_tiles[c][:, :]
            # ones columns (col 65h+64) provide softmax sums in attnV
            for c in range(NT):
                nc.gpsimd.memset(
                    vslice(c).rearrange("p (h w) -> p h w", w=65)[:, :, 64:65],
                    1.0,
                )

            with (
                P(name="wvp", bufs=1) as wvp,
                P(name="vstr", bufs=5) as vstr,
                P(name="psV", bufs=3, space="PSUM") as psV,
            ):
                wv_sb = wvp.tile([128, NK * D_MODEL], BF16, tag="wvsb")
                nc.scalar.dma_start(wv_sb[:, 0:4096], wv[:, 0:4096])
                nc.gpsimd.dma_start(wv_sb[:, 4096:8192], wv[:, 4096:8192])
                for c in range(NT):
                    vts = vstr.tile([128, 1024], BF16, tag="vts")
                    nc.sync.dma_start(vts[:], vt[c])
                    ps = psV.tile([128, 1024], F32, tag="vproj")
                    for k in range(NK):
                        for j in range(2):
                            for r0, r1 in _rhs():
                                nc.tensor.matmul(
                                    ps[:, 512 * j : 512 * j + 512],
                                    vts[r0:r1, 128 * k : 128 * k + 128],
                                    wv_sb[
                                        r0:r1,
                                        1024 * k + 512 * j : 1024 * k + 512 * j + 512,
                                    ],
                                    start=(k == 0 and r0 == 0),
                                    stop=(k == NK - 1 and r1 == 128),
                                    skip_group_check=True,
                                )
                    dst = vslice(c).rearrange("p (h w) -> p h w", w=65)[:, :, 0:64]
                    with nc.allow_low_precision(
                        reason="fp8 V for DoubleRow attnV; softmax-averaged"
                    ):
                        nc.vector.tensor_tensor(
                            dst,
                            ps[:, :].rearrange("p (h w) -> p h w", w=64),
                            bvr_t[:, :].rearrange("p (h w) -> p h w", w=64),
                            ADD,
                        )

            # ---- K/Q projections interleaved into attention --------------
            xn_tiles = [None] * NPAIR
            xg_tiles = [None] * NPAIR
            KT = [
                pkq.tile([128, S], BF16, name=f"ktg{g}", tag=f"ktg{g}")
                for g in range(NPAIR)
            ]
            QT = [
                pkq.tile([128, QL], BF16, name=f"qtg{g}", tag=f"qtg{g}")
                for g in range(NPAIR)
            ]

            import contextlib

            def emit_kproj(half, g, wkg, sc_pool=False):
                # KT[g][:, 1024*half:+1024] = (wk[g].T @ K^T)(half) + bias
                if sc_pool:
                    ps = psS.tile([128, QL], F32, name="ps0", tag="scores")
                else:
                    ps = psP.tile([128, 1024], F32, tag="kproj")
                for k in range(NK):
                    for j in range(2):
                        for r0, r1 in _rhs():
                            nc.tensor.matmul(
                                ps[:, 512 * j : 512 * j + 512],
                                wkg[r0:r1, 128 * k : 128 * k + 128],
                                kt_sb[
                                    r0:r1,
                                    2048 * k
                                    + 1024 * half
                                    + 512 * j : 2048 * k
                                    + 1024 * half
                                    + 512 * j
                                    + 512,
                                ],
                                start=(k == 0 and r0 == 0),
                                stop=(k == NK - 1 and r1 == 128),
                                skip_group_check=True,
                            )
                nc.vector.tensor_scalar_add(
                    KT[g][:, 1024 * half : 1024 * half + 1024],
                    ps[:],
                    bkt_t[:, g : g + 1],
                )

            def emit_qproj(g, wqg, sc_pool=False):
                if sc_pool:
                    ps = psS.tile([128, QL], F32, name="ps0", tag="scores")
                else:
                    ps = psP.tile([128, 1024], F32, tag="kproj")
                for k in range(NK):
                    for j in range(2):
                        for r0, r1 in _rhs():
                            nc.tensor.matmul(
                                ps[:, 512 * j : 512 * j + 512],
                                wqg[r0:r1, 128 * k : 128 * k + 128],
                                qt_sb[
                                    r0:r1,
                                    1024 * k + 512 * j : 1024 * k + 512 * j + 512,
                                ],
                                start=(k == 0 and r0 == 0),
                                stop=(k == NK - 1 and r1 == 128),
                                skip_group_check=True,
                            )
                nc.vector.tensor_scalar_add(QT[g][:], ps[:], bqt_t[:, g : g + 1])

            def load_wk(g):
                wkg = wks.tile([128, D_MODEL], BF16, tag="wks")
                nc.gpsimd.dma_start(wkg[:], wk[g])
                return wkg

            def load_wq(g):
                wqg = wqs.tile([128, D_MODEL], BF16, tag="wqs")
                nc.gpsimd.dma_start(wqg[:], wq[g])
                return wqg

            def proj_unit_gen():
                # pairs g>=1, deferred into the attention loop
                for g in range(1, NPAIR):
                    wkg = load_wk(g)
                    yield emit_kproj, (0, g, wkg)
                    yield emit_kproj, (1, g, wkg)
                    wqg = load_wq(g)
                    yield emit_qproj, (g, wqg)
                kq_es.close()

            with (
                P(name="expp", bufs=6) as expp,
                P(name="psS", bufs=2, space="PSUM") as psS,
                P(name="psacc", bufs=1, space="PSUM") as psacc,
                P(name="psP", bufs=1, space="PSUM") as psP,
                P(name="sgx", bufs=2) as sgxp,
                P(name="pxg", bufs=1) as pxg,
            ):
                # kq pools opened inside the attention pool scope so they
                # can be closed (and their SBUF reused) once the last
                # projection has been emitted
                kq_es = contextlib.ExitStack()
                kstr = kq_es.enter_context(P(name="kstr", bufs=1))
                qstr = kq_es.enter_context(P(name="qstr", bufs=1))
                wks = kq_es.enter_context(P(name="wks", bufs=2))
                wqs = kq_es.enter_context(P(name="wqs", bufs=2))

                kt_sb = kstr.tile([128, NK * S], BF16, tag="ktsb")
                nc.gpsimd.dma_start(kt_sb[:], kt[:])
                qt_sb = qstr.tile([128, NK * QL], BF16, tag="qtsb")
                nc.scalar.dma_start(qt_sb[:], qt[:])

                # pair 0 projections upfront
                wkg0 = load_wk(0)
                emit_kproj(0, 0, wkg0, sc_pool=True)
                emit_kproj(1, 0, wkg0, sc_pool=True)
                wqg0 = load_wq(0)
                emit_qproj(0, wqg0, sc_pool=True)

                proj_iter = proj_unit_gen()

                def emit_scores(g, qb, cg):
                    ktg, qtg = KT[g], QT[g]
                    q0 = 512 * qb
                    tiles = [
                        psS.tile([128, QL], F32, name=f"sc{h}", tag="scores")
                        for h in range(2)
                    ]
                    # alternate row groups (h) so each LDW overlaps the
                    # other head's stream
                    for ci in range(2):
                        c = 2 * cg + ci
                        for h in range(2):
                            p0 = 64 * h
                            nc.tensor.matmul(
                                tiles[h][:, 512 * ci : 512 * ci + 512],
                                ktg[p0 : p0 + 64, 128 * c : 128 * c + 128],
                                qtg[p0 : p0 + 64, q0 : q0 + 512],
                                start=True,
                                stop=True,
                                skip_group_check=True,
                            )
                    return tiles

                def emit_spill(g, qb, acc):
                    # X rows -> xg (bf16, SBUF); recip of sum row -> srs stage
                    for h in range(2):
                        if h == 0:
                            # partitions line up: copy straight into xg
                            nc.vector.tensor_copy(
                                xg_tiles[g][0:64, 512 * qb : 512 * qb + 512],
                                acc[h][0:64, :],
                            )
                        else:
                            sx = sgxp.tile([64, 512], BF16, tag="sgx")
                            nc.vector.tensor_copy(sx[:], acc[h][0:64, :])
                            nc.sync.dma_start(
                                xg_tiles[g][64:128, 512 * qb : 512 * qb + 512], sx[:]
                            )
                        sd = sgxp.tile([65, 512], F32, tag="sgd")
                        nc.vector.tensor_copy(sd[64:65, :], acc[h][64:65, :])
                        nc.sync.dma_start(
                            den[g][2 * h + qb : 2 * h + qb + 1, :], sd[64:65, :]
                        )

                def emit_chain(g):
                    # denr = 1/den ; rep = sel.T @ denr (fp16 broadcast MM)
                    with nc.allow_low_precision(reason="fp16 softmax recips"):
                        nc.vector.reciprocal(denr[g][:], den[g][:])
                    rep = psP.tile([128, QL], F32, tag="kproj")
                    for qb in range(2):
                        nc.tensor.matmul(
                            rep[:, 512 * qb : 512 * qb + 512],
                            sel[qb][:],
                            denr[g][:],
                            start=True,
                            stop=True,
                            skip_group_check=True,
                        )
                    xn = pkq.tile([128, QL], BF16, name=f"xn{g}", tag=f"xn{g}")
                    nc.vector.tensor_tensor(xn[:], xg_tiles[g][:], rep[:], MULT)
                    xn_tiles[g] = xn

                pending_spill = None
                pending_chain = None
                slot = 0
                for g in range(NPAIR):
                    xg_tiles[g] = pxg.tile(
                        [128, QL], BF16, name=f"xg{g}", tag=f"xg{g}"
                    )
                    den[g] = denp.tile([4, 512], F32, name=f"den{g}", tag="den")
                    denr[g] = denp.tile([4, 512], F16, name=f"denr{g}", tag="denr")
                    for qb in range(2):
                        acc = [
                            psacc.tile([65, 512], F32, name="acca", tag="acca"),
                            psacc.tile([65, 512], F32, name="accb", tag="accb"),
                        ]
                        sc_cur = emit_scores(g, qb, 0)
                        if pending_spill is not None:
                            emit_spill(*pending_spill)
                            pending_spill = None
                            if g >= 1 and qb == 1:
                                # spills of (g, qb=0,1) for pair g-? -> chain
                                pending_chain = g - 1
                        for cg in range(NT // 2):
                            sc_next = (
                                emit_scores(g, qb, cg + 1)
                                if cg + 1 < NT // 2
                                else None
                            )
                            for h in range(2):
                                hh = 2 * g + h
                                if ATTNV_FP8:
                                    # exp(s/8 - 3): shift keeps ex under the
                                    # fp8e4 max (+-240); cancels in X/d
                                    ex = expp.tile([128, QL], FP8, tag="exp")
                                    with nc.allow_low_precision(
                                        reason="fp8 softmax weights"
                                    ):
                                        nc.scalar.activation(
                                            ex[:],
                                            sc_cur[h][:],
                                            AF.Exp,
                                            scale=0.125,
                                            bias=expb[:],
                                        )
                                    nc.tensor.matmul(
                                        acc[h][:],
                                        vp_tiles[cg][:, :].rearrange(
                                            "p (c w) -> p c w", c=2
                                        )[:, :, 65 * hh : 65 * hh + 65],
                                        ex[:, :].rearrange(
                                            "p (c q) -> p c q", c=2
                                        ),
                                        start=(cg == 0),
                                        stop=(cg == NT // 2 - 1),
                                        perf_mode=mybir.MatmulPerfMode.DoubleRow,
                                        skip_group_check=True,
                                    )
                                else:
                                    ex = expp.tile([128, QL], BF16, tag="exp")
                                    nc.scalar.activation(
                                        ex[:], sc_cur[h][:], AF.Exp, scale=0.125
                                    )
                                    for ci in range(2):
                                        c = 2 * cg + ci
                                        for r0, r1 in _rhs():
                                            nc.tensor.matmul(
                                                acc[h][:],
                                                v_tiles[c][
                                                    r0:r1,
                                                    65 * hh : 65 * hh + 65,
                                                ],
                                                ex[
                                                    r0:r1,
                                                    512 * ci : 512 * ci + 512,
                                                ],
                                                start=(c == 0 and r0 == 0),
                                                stop=(c == NT - 1 and r1 == 128),
                                                skip_group_check=True,
                                            )
                            if slot % 4 == 1:
                                step = next(proj_iter, None)
                                if step is not None:
                                    step[0](*step[1])
                            if pending_chain is not None and cg == 2:
                                emit_chain(pending_chain)
                                pending_chain = None
                            slot += 1
                            sc_cur = sc_next
                        pending_spill = (g, qb, acc)
                emit_spill(*pending_spill)
                for step in proj_iter:
                    step[0](*step[1])
                emit_chain(7)

            # ---- output projection ---------------------------------------
            with (
                P(name="pwo", bufs=1) as pwo,
                P(name="ps3o", bufs=4, space="PSUM") as ps3o,
            ):
            bor_t = consts.tile([128, D_MODEL], BF16, tag="bor")
            nc.scalar.dma_start(bor_t[:], bor[:])
            wo_sb = pwo.tile([128, NPAIR * D_MODEL], BF16, tag="wosb")
            nc.scalar.dma_start(wo_sb[:], wo[:])

            qrr = 0
            for m in range(QL // 128):
                for j in range(2):
                    ps = ps3o.tile([128, 512], F32, tag="oproj")
                    for g in range(NPAIR):
                        for r0, r1 in _rhs():
                            nc.tensor.matmul(
                                ps[:],
                                xn_tiles[g][
                                    r0:r1, 128 * m : 128 * m + 128
                                ],
                                wo_sb[
                                    r0:r1,
                                    1024 * g + 512 * j : 1024 * g + 512 * j + 512,
                                ],
                                start=(g == 0 and r0 == 0),
                                stop=(g == NPAIR - 1 and r1 == 128),
                                skip_group_check=True,
                            )
                    ot = stg.tile([128, 512], F32, tag="outs")
                    nc.vector.tensor_tensor(
                        ot[:], ps[:], bor_t[:, 512 * j : 512 * j + 512], ADD
                    )
                    eng = (nc.sync, nc.scalar, nc.gpsimd)[qrr % 3]
                    qrr += 1
                    eng.dma_start(
                        out[128 * m : 128 * m + 128, 512 * j : 512 * j + 512], ot[:]
                    )


_NC_CACHE = None
LAST_RESULT = None


def _get_nc():
    global _NC_CACHE
    if _NC_CACHE is None:
        _install_patch()
        _NC_CACHE = _build_bass()
    return _NC_CACHE


def kernel(q, k, v, w_q, b_q, w_k, b_k, w_v, b_v, w_o, b_o):
    global LAST_RESULT
    import ml_dtypes

    q = np.asarray(q, np.float32)
    k = np.asarray(k, np.float32)
    v = np.asarray(v, np.float32)

    def _pair_w(w):
        # [in, out] -> [g, 128, 1024]: [g][p, 128k+j] = w[128k+p, 128g+j]
        return np.ascontiguousarray(
            np.asarray(w, np.float32)
            .reshape(NK, 128, NPAIR, 128)
            .transpose(2, 1, 0, 3)
            .reshape(NPAIR, 128, D_MODEL)
        ).astype(ml_dtypes.bfloat16)

    def _chunk_w(w):
        # [in, out] -> [128, 8*1024]: [p, 1024k+o] = w[128k+p, o]
        return np.ascontiguousarray(
            np.asarray(w, np.float32)
            .reshape(NK, 128, D_MODEL)
            .transpose(1, 0, 2)
            .reshape(128, NK * D_MODEL)
        ).astype(ml_dtypes.bfloat16)

    w_q = _pair_w(w_q)
    w_k = _pair_w(w_k)
    w_v = _chunk_w(w_v)
    # wo: [p, 1024g+o] = w_o[128g+p, o] -- same transform (g indexes chunks)
    w_o = _chunk_w(w_o)
    b_q = np.asarray(b_q, np.float32)
    b_k = np.asarray(b_k, np.float32)
    b_v = np.asarray(b_v, np.float32)
    b_o = np.asarray(b_o, np.float32)

    bqt = np.ascontiguousarray(b_q.reshape(NK, 128).T)
    bkt = np.ascontiguousarray(b_k.reshape(NK, 128).T)
    bvr = np.ascontiguousarray(
        np.broadcast_to(b_v[None, :], (128, D_MODEL))
    ).astype(ml_dtypes.bfloat16)
    bor = np.ascontiguousarray(
        np.broadcast_to(b_o[None, :], (128, D_MODEL))
    ).astype(ml_dtypes.bfloat16)
    selq = np.zeros((2, 4, 128), np.float16)
    for qb in range(2):
        selq[qb, qb, 0:64] = 1.0
        selq[qb, 2 + qb, 64:128] = 1.0

    in_maps = []
    for c in range(N_CORES):
        b = c // 2
        r0 = QL * (c % 2)
        # qt: [p, 1024k+t] = q_proj_input^T chunked
        qtc = np.ascontiguousarray(
            q[b, r0 : r0 + QL, :].T.reshape(NK, 128, QL).transpose(1, 0, 2).reshape(
                128, NK * QL
            )
        ).astype(ml_dtypes.bfloat16)
        ktc = np.ascontiguousarray(
            k[b].T.reshape(NK, 128, S).transpose(1, 0, 2).reshape(128, NK * S)
        ).astype(ml_dtypes.bfloat16)
        in_maps.append(
            {
                "qt": qtc,
                "kt": ktc,
                "vt": np.ascontiguousarray(
                    v[b]
                    .T.reshape(8, 128, 16, 128)
                    .transpose(2, 1, 0, 3)
                    .reshape(16, 128, 1024)
                ).astype(ml_dtypes.bfloat16),
                "wq": w_q,
                "wk": w_k,
                "wv": w_v,
                "wo": w_o,
                "bqt": bqt,
                "bkt": bkt,
                "bvr": bvr,
                "bor": bor,
                "selq": selq,
            }
        )

    nc = _get_nc()
    res = run_bass_kernel_spmd(nc, in_maps, list(range(N_CORES)))
    LAST_RESULT = res

    outp = np.empty((B, S, D_MODEL), np.float32)
    for c in range(N_CORES):
        b = c // 2
        r0 = QL * (c % 2)
        outp[b, r0 : r0 + QL, :] = res.results[c]["out"]
    return outp
